# revision 21
# baseline (speedup 1.0000x reference)
"""GQA attention (RoPE, causal) + output projection for Trainium2, 8 NeuronCores.

Problem: B=2, T=2048, HID=2048, NH=16 Q-heads, NKV=4 KV-heads, HD=128.
Sharding: tensor-parallel over the 4 KV-head groups (4 Q heads + 1 KV head per
group) x data-parallel over batch (2). Core c handles batch c//4, group c%4.
Each core computes its group's partial output y_g = A_g @ Wo[rows_g]; the
host unshards by summing the 4 row-parallel partials per batch.

The x shard is laid out transposed ([HID, T]) at shard-prep time so the
contraction dim lands on SBUF partitions without any on-device transposes.

Per-core device pipeline (all matmuls bf16, f32 accumulation in PSUM):
  A. Projections produce Q^T/K^T [d, t] directly (lhsT=W, rhs=xT) and
     V^T -> XBAR-transposed to natural [t, d]. RoPE is applied in [d, t]
     layout: rotate-half = two cross-partition DVE copies, tables arrive
     host-transposed; 1/sqrt(HD) is folded into the Q tables.
  B. Scores transposed: ST[kv,q] = matmul(lhsT=kT chunk, rhs=qT), exp on
     ScalarE (scores ~N(0,1): no max subtraction needed), multiplicative
     bf16 causal mask on diagonal tiles, then AT[d,q] += matmul(lhsT=V
     chunk, rhs=expST). Softmax sums via GpSimd partition-reduce of expST,
     reciprocal on DVE, GpSimd partition-broadcast, one DVE mul -> aT.
  C. y = A @ Wo via lhsT=aT slices, rhs=Wo; PSUM->SBUF copy on ScalarE.
"""

import numpy as np
import ml_dtypes

import concourse.bass as bass
import concourse.mybir as mybir
import concourse.tile as tile
from concourse import bacc
from concourse.bass_utils import run_bass_kernel_spmd

B, T, HID = 2, 2048, 2048
NH, NKV = 16, 4
HD = 128
GROUPS = NH // NKV      # 4 q-heads per kv head
NQ = GROUPS             # q heads per core
QW = NQ * HD            # 512 q cols per core
P = 128
TB = T // P             # 16 t-blocks
HC = HID // P           # 16 hid chunks
QS = T // 512           # 4 q supertiles
KVC = T // P            # 16 kv chunks
TS = T // 512           # 4 t supertiles
ROPE_BASE = 10000.0

F32 = mybir.dt.float32
BF16 = mybir.dt.bfloat16
EXP = mybir.ActivationFunctionType.Exp


def build_nc():
    nc = bacc.Bacc("TRN2", target_bir_lowering=False, debug=False,
                   enable_asserts=False, num_devices=8)

    xT_d = nc.dram_tensor("xT", [HID, T], F32, kind="ExternalInput")
    wq_d = nc.dram_tensor("wq", [HID, QW], F32, kind="ExternalInput")
    wk_d = nc.dram_tensor("wk", [HID, HD], F32, kind="ExternalInput")
    wv_d = nc.dram_tensor("wv", [HID, HD], F32, kind="ExternalInput")
    wo_d = nc.dram_tensor("wo", [QW, HID], F32, kind="ExternalInput")
    cosq_d = nc.dram_tensor("cosqT", [HD, T], BF16, kind="ExternalInput")
    sinq_d = nc.dram_tensor("sinqT", [HD, T], BF16, kind="ExternalInput")
    cosk_d = nc.dram_tensor("coskT", [HD, T], BF16, kind="ExternalInput")
    sink_d = nc.dram_tensor("sinkT", [HD, T], BF16, kind="ExternalInput")
    masks_d = nc.dram_tensor("masks", [4, P, 512], BF16, kind="ExternalInput")
    y_d = nc.dram_tensor("y", [T, HID], F32, kind="ExternalOutput")

    with tile.TileContext(nc) as tc:
        with tc.tile_pool(name="persist", bufs=1) as persist:
            # ---- persistent SBUF ----
            qT = persist.tile([P, NQ, T], BF16)        # (d, h, t)
            kT = persist.tile([P, T], BF16)            # (d, t)
            vnat = persist.tile([P, KVC, HD], BF16)    # (t, kvc, d)
            aT = persist.tile([P, NQ, T], BF16)        # (d, h, t)
            wq_s = persist.tile([P, HC, NQ, HD], BF16)
            wk_s = persist.tile([P, HC, HD], BF16)
            wv_s = persist.tile([P, HC, HD], BF16)
            wo_s = persist.tile([P, NQ, HID], BF16)
            cq_s = persist.tile([P, T], BF16)
            sq_s = persist.tile([P, T], BF16)
            ck_s = persist.tile([P, T], BF16)
            sk_s = persist.tile([P, T], BF16)
            masks_s = persist.tile([P, 4, 512], BF16)

            # ---- weights/constants load + cast (ordered so the first
            # projections' inputs arrive earliest: wk/wv -> tables -> wq) ----
            wpool_ctx = tc.tile_pool(name="stageW", bufs=1)
            stageW = wpool_ctx.__enter__()
            wkf = stageW.tile([P, HC, HD], F32, tag="wkf")
            nc.sync.dma_start(wkf[:], wk_d.ap().rearrange("(hc p) d -> p hc d", p=P))
            nc.vector.tensor_copy(wk_s[:], wkf[:])
            wvf = stageW.tile([P, HC, HD], F32, tag="wvf")
            nc.sync.dma_start(wvf[:], wv_d.ap().rearrange("(hc p) d -> p hc d", p=P))
            nc.vector.tensor_copy(wv_s[:], wvf[:])
            nc.sync.dma_start(masks_s[:], masks_d.ap().rearrange("o p q -> p o q"))
            nc.sync.dma_start(cq_s[:], cosq_d[:])
            nc.sync.dma_start(sq_s[:], sinq_d[:])
            nc.sync.dma_start(ck_s[:], cosk_d[:])
            nc.sync.dma_start(sk_s[:], sink_d[:])
            wpool_ctx.__exit__(None, None, None)

            # ---- stage A: projections + RoPE, per t-supertile ----
            with (
                tc.tile_pool(name="psA", bufs=2, space="PSUM") as psA,
                tc.tile_pool(name="stageA", bufs=3) as stageA,
            ):
                for ts in range(TS):
                    t0 = ts * 512
                    xts = stageA.tile([P, HC, 512], BF16, tag="xts", bufs=3)
                    for hq in range(4):
                        xf = stageA.tile([P, 4, 512], F32, tag="xf", bufs=2)
                        dma_eng = nc.sync if hq % 2 == 0 else nc.scalar
                        dma_eng.dma_start(
                            xf[:],
                            xT_d.ap()[hq * 4 * P:(hq + 1) * 4 * P, t0:t0 + 512]
                            .rearrange("(hc p) t -> p hc t", p=P))
                        dst = xts[:, hq * 4:(hq + 1) * 4].rearrange(
                            "p hc t -> p (hc t)")
                        srcv = xf.rearrange("p hc t -> p (hc t)")
                        if hq % 2 == 0:
                            nc.scalar.copy(dst, srcv)
                        else:
                            nc.vector.tensor_copy(dst, srcv)

                    if ts == 0:
                        for wc in range(4):
                            wqf = stageA.tile([P, 4, QW], F32, tag="wqf",
                                              bufs=2)
                            nc.scalar.dma_start(
                                wqf[:],
                                wq_d.ap()[wc * 4 * P:(wc + 1) * 4 * P, :]
                                .rearrange("(hc p) c -> p hc c", p=P))
                            nc.vector.tensor_copy(
                                wq_s[:, wc * 4:(wc + 1) * 4].rearrange(
                                    "p hc c d -> p (hc c d)"),
                                wqf.rearrange("p hc c -> p (hc c)"))

                    def rope(ps, cs, ss, out_slice):
                        rot = stageA.tile([P, 512], F32, tag="rot", bufs=3)
                        nc.vector.tensor_copy(rot[0:64, :], ps[64:128, :])
                        nc.vector.tensor_copy(rot[64:128, :], ps[0:64, :])
                        qc = stageA.tile([P, 512], F32, tag="qc", bufs=3)
                        nc.vector.tensor_mul(qc[:], ps[:], cs)
                        nc.vector.tensor_mul(rot[:], rot[:], ss)
                        nc.vector.tensor_add(out_slice, qc[:], rot[:])

                    k_ps = psA.tile([P, 512], F32, tag="kps")
                    for hc in range(HC):
                        nc.tensor.matmul(k_ps[:], wk_s[:, hc], xts[:, hc],
                                         start=(hc == 0), stop=(hc == HC - 1))
                    rope(k_ps, ck_s[:, t0:t0 + 512], sk_s[:, t0:t0 + 512],
                         kT[:, t0:t0 + 512])
                    v_ps = psA.tile([P, 512], F32, tag="vps")
                    for hc in range(HC):
                        nc.tensor.matmul(v_ps[:], wv_s[:, hc], xts[:, hc],
                                         start=(hc == 0), stop=(hc == HC - 1))
                    vtb = stageA.tile([P, 512], BF16, tag="vtb", bufs=2)
                    nc.scalar.copy(vtb[:], v_ps[:])
                    for j in range(4):
                        nc.sync.dma_start_transpose(
                            vnat[:, ts * 4 + j, :], vtb[:, j * P:(j + 1) * P])
                    for h in range(NQ):
                        q_ps = psA.tile([P, 512], F32, tag="qps")
                        for hc in range(HC):
                            nc.tensor.matmul(q_ps[:], wq_s[:, hc, h],
                                             xts[:, hc],
                                             start=(hc == 0), stop=(hc == HC - 1))
                        rope(q_ps, cq_s[:, t0:t0 + 512], sq_s[:, t0:t0 + 512],
                             qT[:, h, t0:t0 + 512])

            # ---- stage B: attention (heads in pairs to fit 8 PSUM banks) ----
            ones_s = persist.tile([P, P], BF16)
            nc.vector.memset(ones_s[:], 1.0)
            with (
                tc.tile_pool(name="psS", bufs=2, space="PSUM") as psS,
                tc.tile_pool(name="psAv", bufs=1, space="PSUM") as psAv,
                tc.tile_pool(name="stageB", bufs=3) as stageB,
            ):
                for qs in range(QS):
                    # wo load spread across groups, overlapping attention
                    cc = qs
                    wtmp3 = stageB.tile([P, HID], F32, tag="wtmp3", bufs=2)
                    nc.sync.dma_start(wtmp3[:], wo_d[cc * P:(cc + 1) * P, :])
                    nc.scalar.copy(wo_s[:, cc], wtmp3[:])
                    q0 = qs * 512
                    nkv = (qs + 1) * 4
                    for hp in range(2):
                        av = psAv.tile([P, 2, 512], F32, tag="av", bufs=2)
                        lb = psAv.tile([P, 2, 512], F32, tag="lb")
                        for kvc in range(nkv):
                            psts = []
                            for hh in range(2):
                                h = hp * 2 + hh
                                st_ps = psS.tile([P, 512], F32, tag="st",
                                                 bufs=2)
                                nc.tensor.matmul(st_ps[:],
                                                 kT[:, kvc * P:(kvc + 1) * P],
                                                 qT[:, h, q0:q0 + 512],
                                                 start=True, stop=True)
                                pst = stageB.tile([P, 512], BF16, tag="pst",
                                                  bufs=4)
                                nc.scalar.activation(pst[:], st_ps[:], EXP)
                                o = kvc - 4 * qs
                                if o >= 0:
                                    nc.vector.tensor_mul(pst[:], pst[:],
                                                         masks_s[:, o, :])
                                psts.append(pst)
                            for hh in range(2):
                                nc.tensor.matmul(av[:, hh], vnat[:, kvc],
                                                 psts[hh][:],
                                                 start=(kvc == 0),
                                                 stop=(kvc == nkv - 1))
                            for hh in range(2):
                                nc.tensor.matmul(lb[:, hh], ones_s[:],
                                                 psts[hh][:],
                                                 start=(kvc == 0),
                                                 stop=(kvc == nkv - 1))
                        for hh in range(2):
                            h = hp * 2 + hh
                            rec = stageB.tile([P, 512], F32, tag="rec", bufs=2)
                            nc.vector.reciprocal_approx_fast(rec[:], lb[:, hh])
                            nc.vector.tensor_mul(aT[:, h, q0:q0 + 512],
                                                 av[:, hh], rec[:])

            # ---- stage C: output projection ----
            with (
                tc.tile_pool(name="psY", bufs=2, space="PSUM") as psY,
                tc.tile_pool(name="stageC", bufs=3) as stageC,
            ):
                for tb in range(TB):
                    y_ps = psY.tile([P, NQ, 512], F32, tag="yps")
                    for ns in range(4):
                        for cc in range(NQ):
                            nc.tensor.matmul(
                                y_ps[:, ns], aT[:, cc, tb * P:(tb + 1) * P],
                                wo_s[:, cc, ns * 512:(ns + 1) * 512],
                                start=(cc == 0), stop=(cc == NQ - 1))
                        y_sb = stageC.tile([P, 512], F32, tag="ysb", bufs=4)
                        if ns % 2 == 0:
                            nc.scalar.copy(y_sb[:], y_ps[:, ns])
                        else:
                            nc.vector.tensor_copy(y_sb[:], y_ps[:, ns])
                        st_eng = nc.sync if ns % 2 == 0 else nc.scalar
                        st_eng.dma_start(
                            y_d[tb * P:(tb + 1) * P, ns * 512:(ns + 1) * 512],
                            y_sb[:])

    nc.compile()
    return nc


def make_tables():
    inv_freq = 1.0 / (ROPE_BASE ** (np.arange(0, HD, 2, dtype=np.float64) / HD))
    t = np.arange(T, dtype=np.float64)
    freqs = np.outer(t, inv_freq)
    emb = np.concatenate([freqs, freqs], axis=-1)        # [T, 128]
    cos = np.cos(emb)
    sin = np.sin(emb)
    sin_signed = sin.copy()
    sin_signed[:, :64] = -sin_signed[:, :64]
    scale = 1.0 / np.sqrt(HD)
    bf = ml_dtypes.bfloat16
    cosqT = np.ascontiguousarray((cos * scale).T).astype(bf)
    sinqT = np.ascontiguousarray((sin_signed * scale).T).astype(bf)
    coskT = np.ascontiguousarray(cos.T).astype(bf)
    sinkT = np.ascontiguousarray(sin_signed.T).astype(bf)
    return cosqT, sinqT, coskT, sinkT


def make_masks():
    # masks[o][i, j] = 1 if (o*128 + i) <= j else 0   (ST tile [kv=128, q=512])
    masks = np.zeros((4, P, 512), dtype=ml_dtypes.bfloat16)
    j = np.arange(512)[None, :]
    i = np.arange(P)[:, None]
    for o in range(4):
        masks[o] = ((o * P + i) <= j).astype(ml_dtypes.bfloat16)
    return masks


def make_in_maps(x, Wq, Wk, Wv, Wo):
    cosqT, sinqT, coskT, sinkT = make_tables()
    masks = make_masks()
    in_maps = []
    for c in range(8):
        b, g = c // 4, c % 4
        in_maps.append({
            "xT": np.ascontiguousarray(x[b].T),
            "wq": np.ascontiguousarray(Wq[:, g * QW:(g + 1) * QW]),
            "wk": np.ascontiguousarray(Wk[:, g * HD:(g + 1) * HD]),
            "wv": np.ascontiguousarray(Wv[:, g * HD:(g + 1) * HD]),
            "wo": np.ascontiguousarray(Wo[g * QW:(g + 1) * QW, :]),
            "cosqT": cosqT, "sinqT": sinqT, "coskT": coskT, "sinkT": sinkT,
            "masks": masks,
        })
    return in_maps


_NC_CACHE = None


def kernel(x, Wq, Wk, Wv, Wo, _trace=False, _tmpdir=None):
    global _NC_CACHE
    x = np.asarray(x, dtype=np.float32)
    Wq = np.asarray(Wq, dtype=np.float32)
    Wk = np.asarray(Wk, dtype=np.float32)
    Wv = np.asarray(Wv, dtype=np.float32)
    Wo = np.asarray(Wo, dtype=np.float32)

    if _NC_CACHE is None:
        _NC_CACHE = build_nc()
    nc = _NC_CACHE

    in_maps = make_in_maps(x, Wq, Wk, Wv, Wo)
    res = run_bass_kernel_spmd(nc, in_maps, core_ids=list(range(8)),
                               trace=_trace, tmpdir=_tmpdir)
    out = np.zeros((B, T, HID), dtype=np.float32)
    for c in range(8):
        out[c // 4] += res.results[c]["y"]
    if _trace:
        return out, res
    return out


# revision 22
# speedup vs baseline: 1.0592x; 1.0592x over previous
"""GQA attention (RoPE, causal) + output projection for Trainium2, 8 NeuronCores.

Problem: B=2, T=2048, HID=2048, NH=16 Q-heads, NKV=4 KV-heads, HD=128.
Sharding: tensor-parallel over the 4 KV-head groups (4 Q heads + 1 KV head per
group) x data-parallel over batch (2). Core c handles batch c//4, group c%4.
Each core computes its group's partial output y_g = A_g @ Wo[rows_g]; the
host unshards by summing the 4 row-parallel partials per batch.

The x shard is laid out transposed ([HID, T]) at shard-prep time so the
contraction dim lands on SBUF partitions without any on-device transposes.

Per-core device pipeline (all matmuls bf16, f32 accumulation in PSUM):
  A. Projections produce Q^T/K^T [d, t] directly (lhsT=W, rhs=xT) and
     V^T -> XBAR-transposed to natural [t, d]. RoPE is applied in [d, t]
     layout: rotate-half = two cross-partition DVE copies, tables arrive
     host-transposed; 1/sqrt(HD) is folded into the Q tables.
  B. Scores transposed: ST[kv,q] = matmul(lhsT=kT chunk, rhs=qT), exp on
     ScalarE (scores ~N(0,1): no max subtraction needed), multiplicative
     bf16 causal mask on diagonal tiles, then AT[d,q] += matmul(lhsT=V
     chunk, rhs=expST). Softmax sums via GpSimd partition-reduce of expST,
     reciprocal on DVE, GpSimd partition-broadcast, one DVE mul -> aT.
  C. y = A @ Wo via lhsT=aT slices, rhs=Wo; PSUM->SBUF copy on ScalarE.
"""

import numpy as np
import ml_dtypes

import concourse.bass as bass
import concourse.mybir as mybir
import concourse.tile as tile
from concourse import bacc
from concourse.bass_utils import run_bass_kernel_spmd

B, T, HID = 2, 2048, 2048
NH, NKV = 16, 4
HD = 128
GROUPS = NH // NKV      # 4 q-heads per kv head
NQ = GROUPS             # q heads per core
QW = NQ * HD            # 512 q cols per core
P = 128
TB = T // P             # 16 t-blocks
HC = HID // P           # 16 hid chunks
QS = T // 512           # 4 q supertiles
KVC = T // P            # 16 kv chunks
TS = T // 512           # 4 t supertiles
ROPE_BASE = 10000.0

F32 = mybir.dt.float32
BF16 = mybir.dt.bfloat16
EXP = mybir.ActivationFunctionType.Exp


def build_nc():
    nc = bacc.Bacc("TRN2", target_bir_lowering=False, debug=False,
                   enable_asserts=False, num_devices=8)

    xT_d = nc.dram_tensor("xT", [HID, T], F32, kind="ExternalInput")
    wq_d = nc.dram_tensor("wq", [HID, QW], F32, kind="ExternalInput")
    wk_d = nc.dram_tensor("wk", [HID, HD], F32, kind="ExternalInput")
    wv_d = nc.dram_tensor("wv", [HID, HD], F32, kind="ExternalInput")
    wo_d = nc.dram_tensor("wo", [QW, HID], F32, kind="ExternalInput")
    cosq_d = nc.dram_tensor("cosqT", [HD, T], BF16, kind="ExternalInput")
    sinq_d = nc.dram_tensor("sinqT", [HD, T], BF16, kind="ExternalInput")
    cosk_d = nc.dram_tensor("coskT", [HD, T], BF16, kind="ExternalInput")
    sink_d = nc.dram_tensor("sinkT", [HD, T], BF16, kind="ExternalInput")
    masks_d = nc.dram_tensor("masks", [4, P, 512], BF16, kind="ExternalInput")
    y_d = nc.dram_tensor("y", [T, HID], BF16, kind="ExternalOutput")

    with tile.TileContext(nc) as tc:
        with tc.tile_pool(name="persist", bufs=1) as persist:
            # ---- persistent SBUF ----
            qT = persist.tile([P, NQ, T], BF16)        # (d, h, t)
            kT = persist.tile([P, T], BF16)            # (d, t)
            vnat = persist.tile([P, KVC, HD], BF16)    # (t, kvc, d)
            aT = persist.tile([P, NQ, T], BF16)        # (d, h, t)
            wq_s = persist.tile([P, HC, NQ, HD], BF16)
            wk_s = persist.tile([P, HC, HD], BF16)
            wv_s = persist.tile([P, HC, HD], BF16)
            wo_s = persist.tile([P, NQ, HID], BF16)
            cq_s = persist.tile([P, T], BF16)
            sq_s = persist.tile([P, T], BF16)
            ck_s = persist.tile([P, T], BF16)
            sk_s = persist.tile([P, T], BF16)
            masks_s = persist.tile([P, 4, 512], BF16)

            # ---- weights/constants load + cast (ordered so the first
            # projections' inputs arrive earliest: wk/wv -> tables -> wq) ----
            wpool_ctx = tc.tile_pool(name="stageW", bufs=1)
            stageW = wpool_ctx.__enter__()
            wkf = stageW.tile([P, HC, HD], F32, tag="wkf")
            nc.sync.dma_start(wkf[:], wk_d.ap().rearrange("(hc p) d -> p hc d", p=P))
            nc.vector.tensor_copy(wk_s[:], wkf[:])
            wvf = stageW.tile([P, HC, HD], F32, tag="wvf")
            nc.sync.dma_start(wvf[:], wv_d.ap().rearrange("(hc p) d -> p hc d", p=P))
            nc.vector.tensor_copy(wv_s[:], wvf[:])
            nc.sync.dma_start(masks_s[:], masks_d.ap().rearrange("o p q -> p o q"))
            nc.sync.dma_start(cq_s[:], cosq_d[:])
            nc.sync.dma_start(sq_s[:], sinq_d[:])
            nc.sync.dma_start(ck_s[:], cosk_d[:])
            nc.sync.dma_start(sk_s[:], sink_d[:])
            wpool_ctx.__exit__(None, None, None)

            # ---- stage A: projections + RoPE, per t-supertile ----
            with (
                tc.tile_pool(name="psA", bufs=2, space="PSUM") as psA,
                tc.tile_pool(name="stageA", bufs=3) as stageA,
            ):
                for ts in range(TS):
                    t0 = ts * 512
                    xts = stageA.tile([P, HC, 512], BF16, tag="xts", bufs=2)
                    for hq in range(4):
                        xf = stageA.tile([P, 4, 512], F32, tag="xf", bufs=3)
                        nc.sync.dma_start(
                            xf[:],
                            xT_d.ap()[hq * 4 * P:(hq + 1) * 4 * P, t0:t0 + 512]
                            .rearrange("(hc p) t -> p hc t", p=P))
                        dst = xts[:, hq * 4:(hq + 1) * 4].rearrange(
                            "p hc t -> p (hc t)")
                        srcv = xf.rearrange("p hc t -> p (hc t)")
                        if hq % 2 == 0:
                            nc.scalar.copy(dst, srcv)
                        else:
                            nc.vector.tensor_copy(dst, srcv)

                    if ts == 0:
                        wqf = stageA.tile([P, HC, QW], F32, tag="wqf", bufs=1)
                        nc.sync.dma_start(
                            wqf[:],
                            wq_d.ap().rearrange("(hc p) c -> p hc c", p=P))
                        nc.vector.tensor_copy(
                            wq_s.rearrange("p hc c d -> p hc (c d)"), wqf[:])

                    def rope(ps, cs, ss, out_slice):
                        rot = stageA.tile([P, 512], F32, tag="rot", bufs=3)
                        nc.vector.tensor_copy(rot[0:64, :], ps[64:128, :])
                        nc.vector.tensor_copy(rot[64:128, :], ps[0:64, :])
                        qc = stageA.tile([P, 512], F32, tag="qc", bufs=3)
                        nc.vector.tensor_mul(qc[:], ps[:], cs)
                        nc.vector.tensor_mul(rot[:], rot[:], ss)
                        nc.vector.tensor_add(out_slice, qc[:], rot[:])

                    k_ps = psA.tile([P, 512], F32, tag="kps")
                    for hc in range(HC):
                        nc.tensor.matmul(k_ps[:], wk_s[:, hc], xts[:, hc],
                                         start=(hc == 0), stop=(hc == HC - 1))
                    rope(k_ps, ck_s[:, t0:t0 + 512], sk_s[:, t0:t0 + 512],
                         kT[:, t0:t0 + 512])
                    v_ps = psA.tile([P, 512], F32, tag="vps")
                    for hc in range(HC):
                        nc.tensor.matmul(v_ps[:], wv_s[:, hc], xts[:, hc],
                                         start=(hc == 0), stop=(hc == HC - 1))
                    vtb = stageA.tile([P, 512], BF16, tag="vtb", bufs=2)
                    nc.scalar.copy(vtb[:], v_ps[:])
                    for j in range(4):
                        nc.sync.dma_start_transpose(
                            vnat[:, ts * 4 + j, :], vtb[:, j * P:(j + 1) * P])
                    for h in range(NQ):
                        q_ps = psA.tile([P, 512], F32, tag="qps")
                        for hc in range(HC):
                            nc.tensor.matmul(q_ps[:], wq_s[:, hc, h],
                                             xts[:, hc],
                                             start=(hc == 0), stop=(hc == HC - 1))
                        rope(q_ps, cq_s[:, t0:t0 + 512], sq_s[:, t0:t0 + 512],
                             qT[:, h, t0:t0 + 512])

            # ---- stage B: attention (heads in pairs to fit 8 PSUM banks) ----
            ones_s = persist.tile([P, P], BF16)
            nc.vector.memset(ones_s[:], 1.0)
            with (
                tc.tile_pool(name="psS", bufs=2, space="PSUM") as psS,
                tc.tile_pool(name="psAv", bufs=1, space="PSUM") as psAv,
                tc.tile_pool(name="stageB", bufs=3) as stageB,
            ):
                for qs in range(QS):
                    # wo load spread across groups, overlapping attention
                    cc = qs
                    wtmp3 = stageB.tile([P, HID], F32, tag="wtmp3", bufs=2)
                    nc.sync.dma_start(wtmp3[:], wo_d[cc * P:(cc + 1) * P, :])
                    nc.scalar.copy(wo_s[:, cc], wtmp3[:])
                    q0 = qs * 512
                    nkv = (qs + 1) * 4
                    for hp in range(2):
                        av = psAv.tile([P, 2, 512], F32, tag="av", bufs=2)
                        lb = psAv.tile([P, 2, 512], F32, tag="lb")
                        for kvc in range(nkv):
                            psts = []
                            for hh in range(2):
                                h = hp * 2 + hh
                                st_ps = psS.tile([P, 512], F32, tag="st",
                                                 bufs=2)
                                nc.tensor.matmul(st_ps[:],
                                                 kT[:, kvc * P:(kvc + 1) * P],
                                                 qT[:, h, q0:q0 + 512],
                                                 start=True, stop=True)
                                pst = stageB.tile([P, 512], BF16, tag="pst",
                                                  bufs=4)
                                nc.scalar.activation(pst[:], st_ps[:], EXP)
                                o = kvc - 4 * qs
                                if o >= 0:
                                    nc.vector.tensor_mul(pst[:], pst[:],
                                                         masks_s[:, o, :])
                                psts.append(pst)
                            for hh in range(2):
                                nc.tensor.matmul(av[:, hh], vnat[:, kvc],
                                                 psts[hh][:],
                                                 start=(kvc == 0),
                                                 stop=(kvc == nkv - 1))
                            for hh in range(2):
                                nc.tensor.matmul(lb[:, hh], ones_s[:],
                                                 psts[hh][:],
                                                 start=(kvc == 0),
                                                 stop=(kvc == nkv - 1))
                        for hh in range(2):
                            h = hp * 2 + hh
                            rec = stageB.tile([P, 512], F32, tag="rec", bufs=2)
                            nc.vector.reciprocal_approx_fast(rec[:], lb[:, hh])
                            nc.vector.tensor_mul(aT[:, h, q0:q0 + 512],
                                                 av[:, hh], rec[:])

            # ---- stage C: output projection ----
            with (
                tc.tile_pool(name="psY", bufs=2, space="PSUM") as psY,
                tc.tile_pool(name="stageC", bufs=3) as stageC,
            ):
                for tb in range(TB):
                    y_ps = psY.tile([P, NQ, 512], F32, tag="yps")
                    for ns in range(4):
                        for cc in range(NQ):
                            nc.tensor.matmul(
                                y_ps[:, ns], aT[:, cc, tb * P:(tb + 1) * P],
                                wo_s[:, cc, ns * 512:(ns + 1) * 512],
                                start=(cc == 0), stop=(cc == NQ - 1))
                        y_sb = stageC.tile([P, 512], BF16, tag="ysb", bufs=4)
                        if ns % 2 == 0:
                            nc.scalar.copy(y_sb[:], y_ps[:, ns])
                        else:
                            nc.vector.tensor_copy(y_sb[:], y_ps[:, ns])
                        st_eng = nc.sync if ns % 2 == 0 else nc.scalar
                        st_eng.dma_start(
                            y_d[tb * P:(tb + 1) * P, ns * 512:(ns + 1) * 512],
                            y_sb[:])

    nc.compile()
    return nc


def make_tables():
    inv_freq = 1.0 / (ROPE_BASE ** (np.arange(0, HD, 2, dtype=np.float64) / HD))
    t = np.arange(T, dtype=np.float64)
    freqs = np.outer(t, inv_freq)
    emb = np.concatenate([freqs, freqs], axis=-1)        # [T, 128]
    cos = np.cos(emb)
    sin = np.sin(emb)
    sin_signed = sin.copy()
    sin_signed[:, :64] = -sin_signed[:, :64]
    scale = 1.0 / np.sqrt(HD)
    bf = ml_dtypes.bfloat16
    cosqT = np.ascontiguousarray((cos * scale).T).astype(bf)
    sinqT = np.ascontiguousarray((sin_signed * scale).T).astype(bf)
    coskT = np.ascontiguousarray(cos.T).astype(bf)
    sinkT = np.ascontiguousarray(sin_signed.T).astype(bf)
    return cosqT, sinqT, coskT, sinkT


def make_masks():
    # masks[o][i, j] = 1 if (o*128 + i) <= j else 0   (ST tile [kv=128, q=512])
    masks = np.zeros((4, P, 512), dtype=ml_dtypes.bfloat16)
    j = np.arange(512)[None, :]
    i = np.arange(P)[:, None]
    for o in range(4):
        masks[o] = ((o * P + i) <= j).astype(ml_dtypes.bfloat16)
    return masks


def make_in_maps(x, Wq, Wk, Wv, Wo):
    cosqT, sinqT, coskT, sinkT = make_tables()
    masks = make_masks()
    in_maps = []
    for c in range(8):
        b, g = c // 4, c % 4
        in_maps.append({
            "xT": np.ascontiguousarray(x[b].T),
            "wq": np.ascontiguousarray(Wq[:, g * QW:(g + 1) * QW]),
            "wk": np.ascontiguousarray(Wk[:, g * HD:(g + 1) * HD]),
            "wv": np.ascontiguousarray(Wv[:, g * HD:(g + 1) * HD]),
            "wo": np.ascontiguousarray(Wo[g * QW:(g + 1) * QW, :]),
            "cosqT": cosqT, "sinqT": sinqT, "coskT": coskT, "sinkT": sinkT,
            "masks": masks,
        })
    return in_maps


_NC_CACHE = None


def kernel(x, Wq, Wk, Wv, Wo, _trace=False, _tmpdir=None):
    global _NC_CACHE
    x = np.asarray(x, dtype=np.float32)
    Wq = np.asarray(Wq, dtype=np.float32)
    Wk = np.asarray(Wk, dtype=np.float32)
    Wv = np.asarray(Wv, dtype=np.float32)
    Wo = np.asarray(Wo, dtype=np.float32)

    if _NC_CACHE is None:
        _NC_CACHE = build_nc()
    nc = _NC_CACHE

    in_maps = make_in_maps(x, Wq, Wk, Wv, Wo)
    res = run_bass_kernel_spmd(nc, in_maps, core_ids=list(range(8)),
                               trace=_trace, tmpdir=_tmpdir)
    out = np.zeros((B, T, HID), dtype=np.float32)
    for c in range(8):
        out[c // 4] += res.results[c]["y"].astype(np.float32)
    if _trace:
        return out, res
    return out


# revision 23
# speedup vs baseline: 1.0963x; 1.0350x over previous
"""GQA attention (RoPE, causal) + output projection for Trainium2, 8 NeuronCores.

Problem: B=2, T=2048, HID=2048, NH=16 Q-heads, NKV=4 KV-heads, HD=128.
Sharding: tensor-parallel over the 4 KV-head groups (4 Q heads + 1 KV head per
group) x data-parallel over batch (2). Core c handles batch c//4, group c%4.
Each core computes its group's partial output y_g = A_g @ Wo[rows_g]; the
host unshards by summing the 4 row-parallel partials per batch.

The x shard is laid out transposed ([HID, T]) at shard-prep time so the
contraction dim lands on SBUF partitions without any on-device transposes.

Per-core device pipeline (all matmuls bf16, f32 accumulation in PSUM):
  A. Projections produce Q^T/K^T [d, t] directly (lhsT=W, rhs=xT) and
     V^T -> XBAR-transposed to natural [t, d]. RoPE is applied in [d, t]
     layout: rotate-half = two cross-partition DVE copies, tables arrive
     host-transposed; 1/sqrt(HD) is folded into the Q tables.
  B. Scores transposed: ST[kv,q] = matmul(lhsT=kT chunk, rhs=qT), exp on
     ScalarE (scores ~N(0,1): no max subtraction needed), multiplicative
     bf16 causal mask on diagonal tiles, then AT[d,q] += matmul(lhsT=V
     chunk, rhs=expST). Softmax sums via GpSimd partition-reduce of expST,
     reciprocal on DVE, GpSimd partition-broadcast, one DVE mul -> aT.
  C. y = A @ Wo via lhsT=aT slices, rhs=Wo; PSUM->SBUF copy on ScalarE.
"""

import numpy as np
import ml_dtypes

import concourse.bass as bass
import concourse.mybir as mybir
import concourse.tile as tile
from concourse import bacc
from concourse.bass_utils import run_bass_kernel_spmd

B, T, HID = 2, 2048, 2048
NH, NKV = 16, 4
HD = 128
GROUPS = NH // NKV      # 4 q-heads per kv head
NQ = GROUPS             # q heads per core
QW = NQ * HD            # 512 q cols per core
P = 128
TB = T // P             # 16 t-blocks
HC = HID // P           # 16 hid chunks
QS = T // 512           # 4 q supertiles
KVC = T // P            # 16 kv chunks
TS = T // 512           # 4 t supertiles
ROPE_BASE = 10000.0

F32 = mybir.dt.float32
BF16 = mybir.dt.bfloat16
EXP = mybir.ActivationFunctionType.Exp


def build_nc():
    nc = bacc.Bacc("TRN2", target_bir_lowering=False, debug=False,
                   enable_asserts=False, num_devices=8)

    xT_d = nc.dram_tensor("xT", [HID, T], F32, kind="ExternalInput")
    wq_d = nc.dram_tensor("wq", [HID, QW], F32, kind="ExternalInput")
    wk_d = nc.dram_tensor("wk", [HID, HD], F32, kind="ExternalInput")
    wv_d = nc.dram_tensor("wv", [HID, HD], F32, kind="ExternalInput")
    wo_d = nc.dram_tensor("wo", [QW, HID], F32, kind="ExternalInput")
    cosq_d = nc.dram_tensor("cosqT", [HD, T], BF16, kind="ExternalInput")
    sinq_d = nc.dram_tensor("sinqT", [HD, T], BF16, kind="ExternalInput")
    cosk_d = nc.dram_tensor("coskT", [HD, T], BF16, kind="ExternalInput")
    sink_d = nc.dram_tensor("sinkT", [HD, T], BF16, kind="ExternalInput")
    masks_d = nc.dram_tensor("masks", [4, P, 512], BF16, kind="ExternalInput")
    y_d = nc.dram_tensor("y", [T, HID], BF16, kind="ExternalOutput")

    with tile.TileContext(nc) as tc:
        with tc.tile_pool(name="persist", bufs=1) as persist:
            # ---- persistent SBUF ----
            qT = persist.tile([P, NQ, T], BF16)        # (d, h, t)
            kT = persist.tile([P, T], BF16)            # (d, t)
            vnat = persist.tile([P, KVC, HD], BF16)    # (t, kvc, d)
            aT = persist.tile([P, NQ, T], BF16)        # (d, h, t)
            wq_s = persist.tile([P, HC, NQ, HD], BF16)
            wk_s = persist.tile([P, HC, HD], BF16)
            wv_s = persist.tile([P, HC, HD], BF16)
            wo_s = persist.tile([P, NQ, HID], BF16)
            cq_s = persist.tile([P, T], BF16)
            sq_s = persist.tile([P, T], BF16)
            ck_s = persist.tile([P, T], BF16)
            sk_s = persist.tile([P, T], BF16)
            masks_s = persist.tile([P, 4, 512], BF16)

            # ---- weights/constants load + cast (ordered so the first
            # projections' inputs arrive earliest: wk/wv -> tables -> wq) ----
            wpool_ctx = tc.tile_pool(name="stageW", bufs=1)
            stageW = wpool_ctx.__enter__()
            wqf = stageW.tile([P, HC, QW], F32, tag="wqf")
            nc.sync.dma_start(wqf[:],
                              wq_d.ap().rearrange("(hc p) c -> p hc c", p=P))
            nc.vector.tensor_copy(
                wq_s.rearrange("p hc c d -> p hc (c d)"), wqf[:])
            wkf = stageW.tile([P, HC, HD], F32, tag="wkf")
            nc.sync.dma_start(wkf[:], wk_d.ap().rearrange("(hc p) d -> p hc d", p=P))
            nc.vector.tensor_copy(wk_s[:], wkf[:])
            wvf = stageW.tile([P, HC, HD], F32, tag="wvf")
            nc.sync.dma_start(wvf[:], wv_d.ap().rearrange("(hc p) d -> p hc d", p=P))
            nc.vector.tensor_copy(wv_s[:], wvf[:])
            nc.sync.dma_start(masks_s[:], masks_d.ap().rearrange("o p q -> p o q"))
            nc.sync.dma_start(cq_s[:], cosq_d[:])
            nc.sync.dma_start(sq_s[:], sinq_d[:])
            nc.sync.dma_start(ck_s[:], cosk_d[:])
            nc.sync.dma_start(sk_s[:], sink_d[:])
            wpool_ctx.__exit__(None, None, None)

            # ---- stage A: projections + RoPE, per t-supertile ----
            with (
                tc.tile_pool(name="psA", bufs=2, space="PSUM") as psA,
                tc.tile_pool(name="stageA", bufs=3) as stageA,
            ):
                for ts in range(TS):
                    t0 = ts * 512
                    xts = stageA.tile([P, HC, 512], BF16, tag="xts", bufs=3)
                    for hq in range(4):
                        xf = stageA.tile([P, 4, 512], F32, tag="xf", bufs=3)
                        nc.sync.dma_start(
                            xf[:],
                            xT_d.ap()[hq * 4 * P:(hq + 1) * 4 * P, t0:t0 + 512]
                            .rearrange("(hc p) t -> p hc t", p=P))
                        dst = xts[:, hq * 4:(hq + 1) * 4].rearrange(
                            "p hc t -> p (hc t)")
                        srcv = xf.rearrange("p hc t -> p (hc t)")
                        if hq % 2 == 0:
                            nc.scalar.copy(dst, srcv)
                        else:
                            nc.vector.tensor_copy(dst, srcv)

                    def rope(ps, cs, ss, out_slice):
                        rot = stageA.tile([P, 512], F32, tag="rot", bufs=3)
                        nc.vector.tensor_copy(rot[0:64, :], ps[64:128, :])
                        nc.vector.tensor_copy(rot[64:128, :], ps[0:64, :])
                        qc = stageA.tile([P, 512], F32, tag="qc", bufs=3)
                        nc.vector.tensor_mul(qc[:], ps[:], cs)
                        nc.vector.tensor_mul(rot[:], rot[:], ss)
                        nc.vector.tensor_add(out_slice, qc[:], rot[:])

                    k_ps = psA.tile([P, 512], F32, tag="kps")
                    for hc in range(HC):
                        nc.tensor.matmul(k_ps[:], wk_s[:, hc], xts[:, hc],
                                         start=(hc == 0), stop=(hc == HC - 1))
                    rope(k_ps, ck_s[:, t0:t0 + 512], sk_s[:, t0:t0 + 512],
                         kT[:, t0:t0 + 512])
                    v_ps = psA.tile([P, 512], F32, tag="vps")
                    for hc in range(HC):
                        nc.tensor.matmul(v_ps[:], wv_s[:, hc], xts[:, hc],
                                         start=(hc == 0), stop=(hc == HC - 1))
                    vtb = stageA.tile([P, 512], BF16, tag="vtb", bufs=2)
                    nc.scalar.copy(vtb[:], v_ps[:])
                    for j in range(4):
                        nc.sync.dma_start_transpose(
                            vnat[:, ts * 4 + j, :], vtb[:, j * P:(j + 1) * P])
                    for h in range(NQ):
                        q_ps = psA.tile([P, 512], F32, tag="qps")
                        for hc in range(HC):
                            nc.tensor.matmul(q_ps[:], wq_s[:, hc, h],
                                             xts[:, hc],
                                             start=(hc == 0), stop=(hc == HC - 1))
                        rope(q_ps, cq_s[:, t0:t0 + 512], sq_s[:, t0:t0 + 512],
                             qT[:, h, t0:t0 + 512])

            # ---- stage B: attention (heads in pairs to fit 8 PSUM banks) ----
            ones_s = persist.tile([P, P], BF16)
            nc.vector.memset(ones_s[:], 1.0)
            with (
                tc.tile_pool(name="psS", bufs=2, space="PSUM") as psS,
                tc.tile_pool(name="psAv", bufs=1, space="PSUM") as psAv,
                tc.tile_pool(name="stageB", bufs=3) as stageB,
            ):
                for qs in range(QS):
                    # wo load spread across groups, overlapping attention
                    cc = qs
                    wtmp3 = stageB.tile([P, HID], F32, tag="wtmp3", bufs=2)
                    nc.sync.dma_start(wtmp3[:], wo_d[cc * P:(cc + 1) * P, :])
                    nc.scalar.copy(wo_s[:, cc], wtmp3[:])
                    q0 = qs * 512
                    nkv = (qs + 1) * 4
                    for hp in range(2):
                        av = psAv.tile([P, 2, 512], F32, tag="av", bufs=2)
                        lb = psAv.tile([P, 2, 512], F32, tag="lb")
                        for kvc in range(nkv):
                            psts = []
                            for hh in range(2):
                                h = hp * 2 + hh
                                st_ps = psS.tile([P, 512], F32, tag="st",
                                                 bufs=2)
                                nc.tensor.matmul(st_ps[:],
                                                 kT[:, kvc * P:(kvc + 1) * P],
                                                 qT[:, h, q0:q0 + 512],
                                                 start=True, stop=True)
                                pst = stageB.tile([P, 512], BF16, tag="pst",
                                                  bufs=4)
                                nc.scalar.activation(pst[:], st_ps[:], EXP)
                                o = kvc - 4 * qs
                                if o >= 0:
                                    nc.vector.tensor_mul(pst[:], pst[:],
                                                         masks_s[:, o, :])
                                psts.append(pst)
                            for hh in range(2):
                                nc.tensor.matmul(av[:, hh], vnat[:, kvc],
                                                 psts[hh][:],
                                                 start=(kvc == 0),
                                                 stop=(kvc == nkv - 1))
                            for hh in range(2):
                                nc.tensor.matmul(lb[:, hh], ones_s[:],
                                                 psts[hh][:],
                                                 start=(kvc == 0),
                                                 stop=(kvc == nkv - 1))
                        for hh in range(2):
                            h = hp * 2 + hh
                            rec = stageB.tile([P, 512], F32, tag="rec", bufs=2)
                            nc.vector.reciprocal_approx_fast(rec[:], lb[:, hh])
                            nc.vector.tensor_mul(aT[:, h, q0:q0 + 512],
                                                 av[:, hh], rec[:])

            # ---- stage C: output projection ----
            with (
                tc.tile_pool(name="psY", bufs=2, space="PSUM") as psY,
                tc.tile_pool(name="stageC", bufs=3) as stageC,
            ):
                for tb in range(TB):
                    y_ps = psY.tile([P, NQ, 512], F32, tag="yps")
                    for npair in range(2):
                        for cc in range(NQ):
                            for ns in (2 * npair, 2 * npair + 1):
                                nc.tensor.matmul(
                                    y_ps[:, ns],
                                    aT[:, cc, tb * P:(tb + 1) * P],
                                    wo_s[:, cc, ns * 512:(ns + 1) * 512],
                                    start=(cc == 0), stop=(cc == NQ - 1))
                    for ns in range(4):
                        y_sb = stageC.tile([P, 512], BF16, tag="ysb", bufs=4)
                        if ns % 2 == 0:
                            nc.scalar.copy(y_sb[:], y_ps[:, ns])
                        else:
                            nc.vector.tensor_copy(y_sb[:], y_ps[:, ns])
                        st_eng = nc.sync if ns % 2 == 0 else nc.scalar
                        st_eng.dma_start(
                            y_d[tb * P:(tb + 1) * P, ns * 512:(ns + 1) * 512],
                            y_sb[:])

    nc.compile()
    return nc


def make_tables():
    inv_freq = 1.0 / (ROPE_BASE ** (np.arange(0, HD, 2, dtype=np.float64) / HD))
    t = np.arange(T, dtype=np.float64)
    freqs = np.outer(t, inv_freq)
    emb = np.concatenate([freqs, freqs], axis=-1)        # [T, 128]
    cos = np.cos(emb)
    sin = np.sin(emb)
    sin_signed = sin.copy()
    sin_signed[:, :64] = -sin_signed[:, :64]
    scale = 1.0 / np.sqrt(HD)
    bf = ml_dtypes.bfloat16
    cosqT = np.ascontiguousarray((cos * scale).T).astype(bf)
    sinqT = np.ascontiguousarray((sin_signed * scale).T).astype(bf)
    coskT = np.ascontiguousarray(cos.T).astype(bf)
    sinkT = np.ascontiguousarray(sin_signed.T).astype(bf)
    return cosqT, sinqT, coskT, sinkT


def make_masks():
    # masks[o][i, j] = 1 if (o*128 + i) <= j else 0   (ST tile [kv=128, q=512])
    masks = np.zeros((4, P, 512), dtype=ml_dtypes.bfloat16)
    j = np.arange(512)[None, :]
    i = np.arange(P)[:, None]
    for o in range(4):
        masks[o] = ((o * P + i) <= j).astype(ml_dtypes.bfloat16)
    return masks


def make_in_maps(x, Wq, Wk, Wv, Wo):
    cosqT, sinqT, coskT, sinkT = make_tables()
    masks = make_masks()
    in_maps = []
    for c in range(8):
        b, g = c // 4, c % 4
        in_maps.append({
            "xT": np.ascontiguousarray(x[b].T),
            "wq": np.ascontiguousarray(Wq[:, g * QW:(g + 1) * QW]),
            "wk": np.ascontiguousarray(Wk[:, g * HD:(g + 1) * HD]),
            "wv": np.ascontiguousarray(Wv[:, g * HD:(g + 1) * HD]),
            "wo": np.ascontiguousarray(Wo[g * QW:(g + 1) * QW, :]),
            "cosqT": cosqT, "sinqT": sinqT, "coskT": coskT, "sinkT": sinkT,
            "masks": masks,
        })
    return in_maps


_NC_CACHE = None


def kernel(x, Wq, Wk, Wv, Wo, _trace=False, _tmpdir=None):
    global _NC_CACHE
    x = np.asarray(x, dtype=np.float32)
    Wq = np.asarray(Wq, dtype=np.float32)
    Wk = np.asarray(Wk, dtype=np.float32)
    Wv = np.asarray(Wv, dtype=np.float32)
    Wo = np.asarray(Wo, dtype=np.float32)

    if _NC_CACHE is None:
        _NC_CACHE = build_nc()
    nc = _NC_CACHE

    in_maps = make_in_maps(x, Wq, Wk, Wv, Wo)
    res = run_bass_kernel_spmd(nc, in_maps, core_ids=list(range(8)),
                               trace=_trace, tmpdir=_tmpdir)
    out = np.zeros((B, T, HID), dtype=np.float32)
    for c in range(8):
        out[c // 4] += res.results[c]["y"].astype(np.float32)
    if _trace:
        return out, res
    return out


# revision 25
# speedup vs baseline: 1.1582x; 1.0565x over previous
"""GQA attention (RoPE, causal) + output projection for Trainium2, 8 NeuronCores.

Problem: B=2, T=2048, HID=2048, NH=16 Q-heads, NKV=4 KV-heads, HD=128.
Sharding: tensor-parallel over the 4 KV-head groups (4 Q heads + 1 KV head per
group) x data-parallel over batch (2). Core c handles batch c//4, group c%4.
Each core computes its group's partial output y_g = A_g @ Wo[rows_g]; the
host unshards by summing the 4 row-parallel partials per batch.

The x shard is laid out transposed ([HID, T]) at shard-prep time so the
contraction dim lands on SBUF partitions without any on-device transposes.

Per-core device pipeline (all matmuls bf16, f32 accumulation in PSUM):
  A. Projections produce Q^T/K^T [d, t] directly (lhsT=W, rhs=xT) and
     V^T -> XBAR-transposed to natural [t, d]. RoPE is applied in [d, t]
     layout: rotate-half = two cross-partition DVE copies, tables arrive
     host-transposed; 1/sqrt(HD) is folded into the Q tables.
  B. Scores transposed: ST[kv,q] = matmul(lhsT=kT chunk, rhs=qT), exp on
     ScalarE (scores ~N(0,1): no max subtraction needed), multiplicative
     bf16 causal mask on diagonal tiles, then AT[d,q] += matmul(lhsT=V
     chunk, rhs=expST). Softmax sums via GpSimd partition-reduce of expST,
     reciprocal on DVE, GpSimd partition-broadcast, one DVE mul -> aT.
  C. y = A @ Wo via lhsT=aT slices, rhs=Wo; PSUM->SBUF copy on ScalarE.
"""

import numpy as np
import ml_dtypes

import concourse.bass as bass
import concourse.mybir as mybir
import concourse.tile as tile
from concourse import bacc
from concourse.bass_utils import run_bass_kernel_spmd

B, T, HID = 2, 2048, 2048
NH, NKV = 16, 4
HD = 128
GROUPS = NH // NKV      # 4 q-heads per kv head
NQ = GROUPS             # q heads per core
QW = NQ * HD            # 512 q cols per core
P = 128
TB = T // P             # 16 t-blocks
HC = HID // P           # 16 hid chunks
QS = T // 512           # 4 q supertiles
KVC = T // P            # 16 kv chunks
TS = T // 512           # 4 t supertiles
ROPE_BASE = 10000.0

F32 = mybir.dt.float32
BF16 = mybir.dt.bfloat16
EXP = mybir.ActivationFunctionType.Exp


def build_nc():
    nc = bacc.Bacc("TRN2", target_bir_lowering=False, debug=False,
                   enable_asserts=False, num_devices=8)

    xT_d = nc.dram_tensor("xT", [HID, T], F32, kind="ExternalInput")
    wq_d = nc.dram_tensor("wq", [HID, QW], F32, kind="ExternalInput")
    wk_d = nc.dram_tensor("wk", [HID, HD], F32, kind="ExternalInput")
    wv_d = nc.dram_tensor("wv", [HID, HD], F32, kind="ExternalInput")
    wo_d = nc.dram_tensor("wo", [QW, HID], F32, kind="ExternalInput")
    cosq_d = nc.dram_tensor("cosqT", [HD, T], BF16, kind="ExternalInput")
    sinq_d = nc.dram_tensor("sinqT", [HD, T], BF16, kind="ExternalInput")
    cosk_d = nc.dram_tensor("coskT", [HD, T], BF16, kind="ExternalInput")
    sink_d = nc.dram_tensor("sinkT", [HD, T], BF16, kind="ExternalInput")
    masks_d = nc.dram_tensor("masks", [P, P], BF16, kind="ExternalInput")
    y_d = nc.dram_tensor("y", [T, HID], BF16, kind="ExternalOutput")

    with tile.TileContext(nc) as tc:
        with tc.tile_pool(name="persist", bufs=1) as persist:
            # ---- persistent SBUF ----
            qT = persist.tile([P, NQ, T], BF16)        # (d, h, t)
            kT = persist.tile([P, T], BF16)            # (d, t)
            vnat = persist.tile([P, KVC, HD], BF16)    # (t, kvc, d)
            aT = persist.tile([P, NQ, T], BF16)        # (d, h, t)
            wq_s = persist.tile([P, HC, NQ, HD], BF16)
            wk_s = persist.tile([P, HC, HD], BF16)
            wv_s = persist.tile([P, HC, HD], BF16)
            wo_s = persist.tile([P, NQ, HID], BF16)
            cq_s = persist.tile([P, T], BF16)
            sq_s = persist.tile([P, T], BF16)
            ck_s = persist.tile([P, T], BF16)
            sk_s = persist.tile([P, T], BF16)
            masks_s = persist.tile([P, P], BF16)

            # ---- weights/constants load + cast (ordered so the first
            # projections' inputs arrive earliest: wk/wv -> tables -> wq) ----
            wpool_ctx = tc.tile_pool(name="stageW", bufs=1)
            stageW = wpool_ctx.__enter__()
            wqf = stageW.tile([P, HC, QW], F32, tag="wqf")
            nc.sync.dma_start(wqf[:],
                              wq_d.ap().rearrange("(hc p) c -> p hc c", p=P))
            nc.vector.tensor_copy(
                wq_s.rearrange("p hc c d -> p hc (c d)"), wqf[:])
            wkf = stageW.tile([P, HC, HD], F32, tag="wkf")
            nc.sync.dma_start(wkf[:], wk_d.ap().rearrange("(hc p) d -> p hc d", p=P))
            nc.vector.tensor_copy(wk_s[:], wkf[:])
            wvf = stageW.tile([P, HC, HD], F32, tag="wvf")
            nc.sync.dma_start(wvf[:], wv_d.ap().rearrange("(hc p) d -> p hc d", p=P))
            nc.vector.tensor_copy(wv_s[:], wvf[:])
            nc.sync.dma_start(masks_s[:], masks_d[:])
            nc.sync.dma_start(cq_s[:], cosq_d[:])
            nc.sync.dma_start(sq_s[:], sinq_d[:])
            nc.sync.dma_start(ck_s[:], cosk_d[:])
            nc.sync.dma_start(sk_s[:], sink_d[:])
            wpool_ctx.__exit__(None, None, None)

            # ---- stage A: projections + RoPE, per t-supertile ----
            with (
                tc.tile_pool(name="psA", bufs=2, space="PSUM") as psA,
                tc.tile_pool(name="stageA", bufs=3) as stageA,
            ):
                for ts in range(TS):
                    t0 = ts * 512
                    xts = stageA.tile([P, HC, 512], BF16, tag="xts", bufs=3)
                    for hq in range(4):
                        xf = stageA.tile([P, 4, 512], F32, tag="xf", bufs=3)
                        nc.sync.dma_start(
                            xf[:],
                            xT_d.ap()[hq * 4 * P:(hq + 1) * 4 * P, t0:t0 + 512]
                            .rearrange("(hc p) t -> p hc t", p=P))
                        dst = xts[:, hq * 4:(hq + 1) * 4].rearrange(
                            "p hc t -> p (hc t)")
                        srcv = xf.rearrange("p hc t -> p (hc t)")
                        if hq % 2 == 0:
                            nc.scalar.copy(dst, srcv)
                        else:
                            nc.vector.tensor_copy(dst, srcv)

                    def rope(ps, cs, ss, out_slice):
                        rot = stageA.tile([P, 512], F32, tag="rot", bufs=3)
                        nc.vector.tensor_copy(rot[0:64, :], ps[64:128, :])
                        nc.vector.tensor_copy(rot[64:128, :], ps[0:64, :])
                        qc = stageA.tile([P, 512], F32, tag="qc", bufs=3)
                        nc.vector.tensor_mul(qc[:], ps[:], cs)
                        nc.vector.tensor_mul(rot[:], rot[:], ss)
                        nc.vector.tensor_add(out_slice, qc[:], rot[:])

                    k_ps = psA.tile([P, 512], F32, tag="kps")
                    for hc in range(HC):
                        nc.tensor.matmul(k_ps[:], wk_s[:, hc], xts[:, hc],
                                         start=(hc == 0), stop=(hc == HC - 1))
                    rope(k_ps, ck_s[:, t0:t0 + 512], sk_s[:, t0:t0 + 512],
                         kT[:, t0:t0 + 512])
                    v_ps = psA.tile([P, 512], F32, tag="vps")
                    for hc in range(HC):
                        nc.tensor.matmul(v_ps[:], wv_s[:, hc], xts[:, hc],
                                         start=(hc == 0), stop=(hc == HC - 1))
                    vtb = stageA.tile([P, 512], BF16, tag="vtb", bufs=2)
                    nc.scalar.copy(vtb[:], v_ps[:])
                    for j in range(4):
                        nc.sync.dma_start_transpose(
                            vnat[:, ts * 4 + j, :], vtb[:, j * P:(j + 1) * P])
                    for h in range(NQ):
                        q_ps = psA.tile([P, 512], F32, tag="qps")
                        for hc in range(HC):
                            nc.tensor.matmul(q_ps[:], wq_s[:, hc, h],
                                             xts[:, hc],
                                             start=(hc == 0), stop=(hc == HC - 1))
                        rope(q_ps, cq_s[:, t0:t0 + 512], sq_s[:, t0:t0 + 512],
                             qT[:, h, t0:t0 + 512])
                    # wo chunk load overlapping later phases
                    wof = stageA.tile([P, HID], F32, tag="wof", bufs=2)
                    nc.sync.dma_start(wof[:], wo_d[ts * P:(ts + 1) * P, :])
                    nc.vector.tensor_copy(wo_s[:, ts], wof[:])

            # ---- stage B: attention (heads in pairs to fit 8 PSUM banks) ----
            ones_s = persist.tile([P, P], BF16)
            nc.vector.memset(ones_s[:], 1.0)
            with (
                tc.tile_pool(name="psS", bufs=2, space="PSUM") as psS,
                tc.tile_pool(name="psAv", bufs=1, space="PSUM") as psAv,
                tc.tile_pool(name="stageB", bufs=3) as stageB,
            ):
                for qs in range(QS):
                    q0 = qs * 512
                    nkv = (qs + 1) * 4
                    for hp in range(2):
                        av = psAv.tile([P, 2, 512], F32, tag="av", bufs=1)
                        laccs = []
                        for hh in range(2):
                            lacc = stageB.tile([P, 512], BF16, bufs=2,
                                               tag=f"lacc{hh}",
                                               name=f"lacc{hh}")
                            laccs.append(lacc)
                        for kvc in range(nkv):
                            o = kvc - 4 * qs
                            c0 = max(o, 0) * P
                            psts = []
                            for hh in range(2):
                                h = hp * 2 + hh
                                st_ps = psS.tile([P, 512], F32, tag="st",
                                                 bufs=2)
                                nc.tensor.matmul(st_ps[:, c0:],
                                                 kT[:, kvc * P:(kvc + 1) * P],
                                                 qT[:, h, q0 + c0:q0 + 512],
                                                 start=True, stop=True)
                                pst = stageB.tile([P, 512], BF16, tag="pst",
                                                  bufs=4)
                                nc.scalar.activation(pst[:, c0:],
                                                     st_ps[:, c0:], EXP)
                                if o >= 0:
                                    nc.vector.tensor_mul(
                                        pst[:, c0:c0 + P], pst[:, c0:c0 + P],
                                        masks_s[:])
                                if kvc == 0:
                                    nc.vector.tensor_copy(laccs[hh][:], pst[:])
                                else:
                                    nc.vector.tensor_add(
                                        laccs[hh][:, c0:], laccs[hh][:, c0:],
                                        pst[:, c0:])
                                psts.append(pst)
                            for hh in range(2):
                                nc.tensor.matmul(av[:, hh, c0:],
                                                 vnat[:, kvc],
                                                 psts[hh][:, c0:],
                                                 start=(kvc == 0),
                                                 stop=(kvc == nkv - 1),
                                                 skip_group_check=True)
                        lb = psAv.tile([P, 2, 512], F32, tag="lb")
                        for hh in range(2):
                            nc.tensor.matmul(lb[:, hh], ones_s[:],
                                             laccs[hh][:],
                                             start=True, stop=True)
                        for hh in range(2):
                            h = hp * 2 + hh
                            rec = stageB.tile([P, 512], F32, tag="rec", bufs=2)
                            nc.vector.reciprocal_approx_fast(rec[:], lb[:, hh])
                            nc.vector.tensor_mul(aT[:, h, q0:q0 + 512],
                                                 av[:, hh], rec[:])
                    # ---- fused output projection for this q-supertile ----
                    for tb in range(4 * qs, 4 * qs + 4):
                        for ns in range(4):
                            y_ps = psAv.tile([P, 512], F32, tag="yps", bufs=2)
                            for cc in range(NQ):
                                nc.tensor.matmul(
                                    y_ps[:], aT[:, cc, tb * P:(tb + 1) * P],
                                    wo_s[:, cc, ns * 512:(ns + 1) * 512],
                                    start=(cc == 0), stop=(cc == NQ - 1))
                            y_sb = stageB.tile([P, 512], BF16, tag="ysb",
                                               bufs=4)
                            if ns % 2 == 0:
                                nc.scalar.copy(y_sb[:], y_ps[:])
                            else:
                                nc.vector.tensor_copy(y_sb[:], y_ps[:])
                            st_eng = nc.sync if ns % 2 == 0 else nc.scalar
                            st_eng.dma_start(
                                y_d[tb * P:(tb + 1) * P,
                                    ns * 512:(ns + 1) * 512],
                                y_sb[:])

    nc.compile()
    return nc


def make_tables():
    inv_freq = 1.0 / (ROPE_BASE ** (np.arange(0, HD, 2, dtype=np.float64) / HD))
    t = np.arange(T, dtype=np.float64)
    freqs = np.outer(t, inv_freq)
    emb = np.concatenate([freqs, freqs], axis=-1)        # [T, 128]
    cos = np.cos(emb)
    sin = np.sin(emb)
    sin_signed = sin.copy()
    sin_signed[:, :64] = -sin_signed[:, :64]
    scale = 1.0 / np.sqrt(HD)
    bf = ml_dtypes.bfloat16
    cosqT = np.ascontiguousarray((cos * scale).T).astype(bf)
    sinqT = np.ascontiguousarray((sin_signed * scale).T).astype(bf)
    coskT = np.ascontiguousarray(cos.T).astype(bf)
    sinkT = np.ascontiguousarray(sin_signed.T).astype(bf)
    return cosqT, sinqT, coskT, sinkT


def make_masks():
    # triangle mask [kv=128, q=128]: 1 where kv_row <= q_col
    j = np.arange(P)[None, :]
    i = np.arange(P)[:, None]
    return (i <= j).astype(ml_dtypes.bfloat16)


def make_in_maps(x, Wq, Wk, Wv, Wo):
    cosqT, sinqT, coskT, sinkT = make_tables()
    masks = make_masks()
    in_maps = []
    for c in range(8):
        b, g = c // 4, c % 4
        in_maps.append({
            "xT": np.ascontiguousarray(x[b].T),
            "wq": np.ascontiguousarray(Wq[:, g * QW:(g + 1) * QW]),
            "wk": np.ascontiguousarray(Wk[:, g * HD:(g + 1) * HD]),
            "wv": np.ascontiguousarray(Wv[:, g * HD:(g + 1) * HD]),
            "wo": np.ascontiguousarray(Wo[g * QW:(g + 1) * QW, :]),
            "cosqT": cosqT, "sinqT": sinqT, "coskT": coskT, "sinkT": sinkT,
            "masks": masks,
        })
    return in_maps


_NC_CACHE = None


def kernel(x, Wq, Wk, Wv, Wo, _trace=False, _tmpdir=None):
    global _NC_CACHE
    x = np.asarray(x, dtype=np.float32)
    Wq = np.asarray(Wq, dtype=np.float32)
    Wk = np.asarray(Wk, dtype=np.float32)
    Wv = np.asarray(Wv, dtype=np.float32)
    Wo = np.asarray(Wo, dtype=np.float32)

    if _NC_CACHE is None:
        _NC_CACHE = build_nc()
    nc = _NC_CACHE

    in_maps = make_in_maps(x, Wq, Wk, Wv, Wo)
    res = run_bass_kernel_spmd(nc, in_maps, core_ids=list(range(8)),
                               trace=_trace, tmpdir=_tmpdir)
    out = np.zeros((B, T, HID), dtype=np.float32)
    for c in range(8):
        out[c // 4] += res.results[c]["y"].astype(np.float32)
    if _trace:
        return out, res
    return out


# revision 26
# speedup vs baseline: 1.1592x; 1.0008x over previous
"""GQA attention (RoPE, causal) + output projection for Trainium2, 8 NeuronCores.

Problem: B=2, T=2048, HID=2048, NH=16 Q-heads, NKV=4 KV-heads, HD=128.
Sharding: tensor-parallel over the 4 KV-head groups (4 Q heads + 1 KV head per
group) x data-parallel over batch (2). Core c handles batch c//4, group c%4.
Each core computes its group's partial output y_g = A_g @ Wo[rows_g]; the
host unshards by summing the 4 row-parallel partials per batch.

The x shard is laid out transposed ([HID, T]) at shard-prep time so the
contraction dim lands on SBUF partitions without any on-device transposes.

Per-core device pipeline (all matmuls bf16, f32 accumulation in PSUM):
  A. Projections produce Q^T/K^T [d, t] directly (lhsT=W, rhs=xT) and
     V^T -> XBAR-transposed to natural [t, d]. RoPE is applied in [d, t]
     layout: rotate-half = two cross-partition DVE copies, tables arrive
     host-transposed; 1/sqrt(HD) is folded into the Q tables.
  B. Scores transposed: ST[kv,q] = matmul(lhsT=kT chunk, rhs=qT), exp on
     ScalarE (scores ~N(0,1): no max subtraction needed), multiplicative
     bf16 causal mask on diagonal tiles, then AT[d,q] += matmul(lhsT=V
     chunk, rhs=expST). Softmax sums via GpSimd partition-reduce of expST,
     reciprocal on DVE, GpSimd partition-broadcast, one DVE mul -> aT.
  C. y = A @ Wo via lhsT=aT slices, rhs=Wo; PSUM->SBUF copy on ScalarE.
"""

import numpy as np
import ml_dtypes

import concourse.bass as bass
import concourse.mybir as mybir
import concourse.tile as tile
from concourse import bacc
from concourse.bass_utils import run_bass_kernel_spmd

B, T, HID = 2, 2048, 2048
NH, NKV = 16, 4
HD = 128
GROUPS = NH // NKV      # 4 q-heads per kv head
NQ = GROUPS             # q heads per core
QW = NQ * HD            # 512 q cols per core
P = 128
TB = T // P             # 16 t-blocks
HC = HID // P           # 16 hid chunks
QS = T // 512           # 4 q supertiles
KVC = T // P            # 16 kv chunks
TS = T // 512           # 4 t supertiles
ROPE_BASE = 10000.0

F32 = mybir.dt.float32
BF16 = mybir.dt.bfloat16
EXP = mybir.ActivationFunctionType.Exp


def build_nc():
    nc = bacc.Bacc("TRN2", target_bir_lowering=False, debug=False,
                   enable_asserts=False, num_devices=8)

    xT_d = nc.dram_tensor("xT", [HID, T], F32, kind="ExternalInput")
    wq_d = nc.dram_tensor("wq", [HID, QW], F32, kind="ExternalInput")
    wk_d = nc.dram_tensor("wk", [HID, HD], F32, kind="ExternalInput")
    wv_d = nc.dram_tensor("wv", [HID, HD], F32, kind="ExternalInput")
    wo_d = nc.dram_tensor("wo", [QW, HID], F32, kind="ExternalInput")
    cosq_d = nc.dram_tensor("cosqT", [HD, T], BF16, kind="ExternalInput")
    sinq_d = nc.dram_tensor("sinqT", [HD, T], BF16, kind="ExternalInput")
    cosk_d = nc.dram_tensor("coskT", [HD, T], BF16, kind="ExternalInput")
    sink_d = nc.dram_tensor("sinkT", [HD, T], BF16, kind="ExternalInput")
    masks_d = nc.dram_tensor("masks", [P, P], BF16, kind="ExternalInput")
    y_d = nc.dram_tensor("y", [T, HID], BF16, kind="ExternalOutput")

    with tile.TileContext(nc) as tc:
        with tc.tile_pool(name="persist", bufs=1) as persist:
            # ---- persistent SBUF ----
            qT = persist.tile([P, NQ, T], BF16)        # (d, h, t)
            kT = persist.tile([P, T], BF16)            # (d, t)
            vnat = persist.tile([P, KVC, HD], BF16)    # (t, kvc, d)
            aT = persist.tile([P, NQ, T], BF16)        # (d, h, t)
            wq_s = persist.tile([P, HC, NQ, HD], BF16)
            wk_s = persist.tile([P, HC, HD], BF16)
            wv_s = persist.tile([P, HC, HD], BF16)
            wo_s = persist.tile([P, NQ, HID], BF16)
            cq_s = persist.tile([P, T], BF16)
            sq_s = persist.tile([P, T], BF16)
            ck_s = persist.tile([P, T], BF16)
            sk_s = persist.tile([P, T], BF16)
            masks_s = persist.tile([P, P], BF16)

            # ---- weights/constants load + cast (ordered so the first
            # projections' inputs arrive earliest: wk/wv -> tables -> wq) ----
            wpool_ctx = tc.tile_pool(name="stageW", bufs=1)
            stageW = wpool_ctx.__enter__()
            wqf = stageW.tile([P, HC, QW], F32, tag="wqf")
            nc.scalar.dma_start(wqf[:],
                                wq_d.ap().rearrange("(hc p) c -> p hc c", p=P))
            nc.vector.tensor_copy(
                wq_s.rearrange("p hc c d -> p hc (c d)"), wqf[:])
            wkf = stageW.tile([P, HC, HD], F32, tag="wkf")
            nc.scalar.dma_start(wkf[:], wk_d.ap().rearrange("(hc p) d -> p hc d", p=P))
            nc.vector.tensor_copy(wk_s[:], wkf[:])
            wvf = stageW.tile([P, HC, HD], F32, tag="wvf")
            nc.scalar.dma_start(wvf[:], wv_d.ap().rearrange("(hc p) d -> p hc d", p=P))
            nc.vector.tensor_copy(wv_s[:], wvf[:])
            nc.scalar.dma_start(masks_s[:], masks_d[:])
            nc.scalar.dma_start(cq_s[:], cosq_d[:])
            nc.scalar.dma_start(sq_s[:], sinq_d[:])
            nc.scalar.dma_start(ck_s[:], cosk_d[:])
            nc.scalar.dma_start(sk_s[:], sink_d[:])
            wpool_ctx.__exit__(None, None, None)

            # ---- stage A: projections + RoPE, per t-supertile ----
            with (
                tc.tile_pool(name="psA", bufs=2, space="PSUM") as psA,
                tc.tile_pool(name="stageA", bufs=3) as stageA,
            ):
                for ts in range(TS):
                    t0 = ts * 512
                    xts = stageA.tile([P, HC, 512], BF16, tag="xts", bufs=3)
                    for hq in range(4):
                        xf = stageA.tile([P, 4, 512], F32, tag="xf", bufs=3)
                        nc.sync.dma_start(
                            xf[:],
                            xT_d.ap()[hq * 4 * P:(hq + 1) * 4 * P, t0:t0 + 512]
                            .rearrange("(hc p) t -> p hc t", p=P))
                        dst = xts[:, hq * 4:(hq + 1) * 4].rearrange(
                            "p hc t -> p (hc t)")
                        srcv = xf.rearrange("p hc t -> p (hc t)")
                        if hq % 2 == 0:
                            nc.scalar.copy(dst, srcv)
                        else:
                            nc.vector.tensor_copy(dst, srcv)

                    def rope(ps, cs, ss, out_slice):
                        rot = stageA.tile([P, 512], F32, tag="rot", bufs=3)
                        nc.vector.tensor_copy(rot[0:64, :], ps[64:128, :])
                        nc.vector.tensor_copy(rot[64:128, :], ps[0:64, :])
                        qc = stageA.tile([P, 512], F32, tag="qc", bufs=3)
                        nc.vector.tensor_mul(qc[:], ps[:], cs)
                        nc.vector.tensor_mul(rot[:], rot[:], ss)
                        nc.vector.tensor_add(out_slice, qc[:], rot[:])

                    k_ps = psA.tile([P, 512], F32, tag="kps")
                    for hc in range(HC):
                        nc.tensor.matmul(k_ps[:], wk_s[:, hc], xts[:, hc],
                                         start=(hc == 0), stop=(hc == HC - 1))
                    rope(k_ps, ck_s[:, t0:t0 + 512], sk_s[:, t0:t0 + 512],
                         kT[:, t0:t0 + 512])
                    v_ps = psA.tile([P, 512], F32, tag="vps")
                    for hc in range(HC):
                        nc.tensor.matmul(v_ps[:], wv_s[:, hc], xts[:, hc],
                                         start=(hc == 0), stop=(hc == HC - 1))
                    vtb = stageA.tile([P, 512], BF16, tag="vtb", bufs=2)
                    nc.scalar.copy(vtb[:], v_ps[:])
                    for j in range(4):
                        nc.sync.dma_start_transpose(
                            vnat[:, ts * 4 + j, :], vtb[:, j * P:(j + 1) * P])
                    for h in range(NQ):
                        q_ps = psA.tile([P, 512], F32, tag="qps")
                        for hc in range(HC):
                            nc.tensor.matmul(q_ps[:], wq_s[:, hc, h],
                                             xts[:, hc],
                                             start=(hc == 0), stop=(hc == HC - 1))
                        rope(q_ps, cq_s[:, t0:t0 + 512], sq_s[:, t0:t0 + 512],
                             qT[:, h, t0:t0 + 512])
                    # wo chunk load overlapping later phases
                    wof = stageA.tile([P, HID], F32, tag="wof", bufs=2)
                    nc.scalar.dma_start(wof[:], wo_d[ts * P:(ts + 1) * P, :])
                    nc.vector.tensor_copy(wo_s[:, ts], wof[:])

            # ---- stage B: attention (heads in pairs to fit 8 PSUM banks) ----
            ones_s = persist.tile([P, P], BF16)
            nc.vector.memset(ones_s[:], 1.0)
            with (
                tc.tile_pool(name="psS", bufs=2, space="PSUM") as psS,
                tc.tile_pool(name="psAv", bufs=1, space="PSUM") as psAv,
                tc.tile_pool(name="stageB", bufs=3) as stageB,
            ):
                for qs in range(QS):
                    q0 = qs * 512
                    nkv = (qs + 1) * 4
                    for hp in range(2):
                        av = psAv.tile([P, 2, 512], F32, tag="av", bufs=1)
                        laccs = []
                        for hh in range(2):
                            lacc = stageB.tile([P, 512], BF16, bufs=2,
                                               tag=f"lacc{hh}",
                                               name=f"lacc{hh}")
                            laccs.append(lacc)
                        for kvc in range(nkv):
                            o = kvc - 4 * qs
                            c0 = max(o, 0) * P
                            psts = []
                            for hh in range(2):
                                h = hp * 2 + hh
                                st_ps = psS.tile([P, 512], F32, tag="st",
                                                 bufs=2)
                                nc.tensor.matmul(st_ps[:, c0:],
                                                 kT[:, kvc * P:(kvc + 1) * P],
                                                 qT[:, h, q0 + c0:q0 + 512],
                                                 start=True, stop=True)
                                pst = stageB.tile([P, 512], BF16, tag="pst",
                                                  bufs=4)
                                nc.scalar.activation(pst[:, c0:],
                                                     st_ps[:, c0:], EXP)
                                if o >= 0:
                                    nc.vector.tensor_mul(
                                        pst[:, c0:c0 + P], pst[:, c0:c0 + P],
                                        masks_s[:])
                                if kvc == 0:
                                    nc.vector.tensor_copy(laccs[hh][:], pst[:])
                                else:
                                    nc.vector.tensor_add(
                                        laccs[hh][:, c0:], laccs[hh][:, c0:],
                                        pst[:, c0:])
                                psts.append(pst)
                            for hh in range(2):
                                nc.tensor.matmul(av[:, hh, c0:],
                                                 vnat[:, kvc],
                                                 psts[hh][:, c0:],
                                                 start=(kvc == 0),
                                                 stop=(kvc == nkv - 1),
                                                 skip_group_check=True)
                        lb = psAv.tile([P, 2, 512], F32, tag="lb")
                        for hh in range(2):
                            nc.tensor.matmul(lb[:, hh], ones_s[:],
                                             laccs[hh][:],
                                             start=True, stop=True)
                        for hh in range(2):
                            h = hp * 2 + hh
                            rec = stageB.tile([P, 512], F32, tag="rec", bufs=2)
                            nc.vector.reciprocal_approx_fast(rec[:], lb[:, hh])
                            nc.vector.tensor_mul(aT[:, h, q0:q0 + 512],
                                                 av[:, hh], rec[:])
                    # ---- fused output projection for this q-supertile ----
                    for tb in range(4 * qs, 4 * qs + 4):
                        for npair in range(2):
                            yp0 = psAv.tile([P, 512], F32, tag="yps", bufs=2,
                                            name="yp0")
                            yp1 = psAv.tile([P, 512], F32, tag="yps", bufs=2,
                                            name="yp1")
                            yps = (yp0, yp1)
                            for cc in range(NQ):
                                for k in range(2):
                                    ns = 2 * npair + k
                                    nc.tensor.matmul(
                                        yps[k][:],
                                        aT[:, cc, tb * P:(tb + 1) * P],
                                        wo_s[:, cc, ns * 512:(ns + 1) * 512],
                                        start=(cc == 0), stop=(cc == NQ - 1))
                            for k in range(2):
                                ns = 2 * npair + k
                                y_sb = stageB.tile([P, 512], BF16, tag="ysb",
                                                   bufs=4)
                                if k == 0:
                                    nc.scalar.copy(y_sb[:], yps[k][:])
                                else:
                                    nc.vector.tensor_copy(y_sb[:], yps[k][:])
                                st_eng = nc.sync if k == 0 else nc.scalar
                                st_eng.dma_start(
                                    y_d[tb * P:(tb + 1) * P,
                                        ns * 512:(ns + 1) * 512],
                                    y_sb[:])

    nc.compile()
    return nc


def make_tables():
    inv_freq = 1.0 / (ROPE_BASE ** (np.arange(0, HD, 2, dtype=np.float64) / HD))
    t = np.arange(T, dtype=np.float64)
    freqs = np.outer(t, inv_freq)
    emb = np.concatenate([freqs, freqs], axis=-1)        # [T, 128]
    cos = np.cos(emb)
    sin = np.sin(emb)
    sin_signed = sin.copy()
    sin_signed[:, :64] = -sin_signed[:, :64]
    scale = 1.0 / np.sqrt(HD)
    bf = ml_dtypes.bfloat16
    cosqT = np.ascontiguousarray((cos * scale).T).astype(bf)
    sinqT = np.ascontiguousarray((sin_signed * scale).T).astype(bf)
    coskT = np.ascontiguousarray(cos.T).astype(bf)
    sinkT = np.ascontiguousarray(sin_signed.T).astype(bf)
    return cosqT, sinqT, coskT, sinkT


def make_masks():
    # triangle mask [kv=128, q=128]: 1 where kv_row <= q_col
    j = np.arange(P)[None, :]
    i = np.arange(P)[:, None]
    return (i <= j).astype(ml_dtypes.bfloat16)


def make_in_maps(x, Wq, Wk, Wv, Wo):
    cosqT, sinqT, coskT, sinkT = make_tables()
    masks = make_masks()
    in_maps = []
    for c in range(8):
        b, g = c // 4, c % 4
        in_maps.append({
            "xT": np.ascontiguousarray(x[b].T),
            "wq": np.ascontiguousarray(Wq[:, g * QW:(g + 1) * QW]),
            "wk": np.ascontiguousarray(Wk[:, g * HD:(g + 1) * HD]),
            "wv": np.ascontiguousarray(Wv[:, g * HD:(g + 1) * HD]),
            "wo": np.ascontiguousarray(Wo[g * QW:(g + 1) * QW, :]),
            "cosqT": cosqT, "sinqT": sinqT, "coskT": coskT, "sinkT": sinkT,
            "masks": masks,
        })
    return in_maps


_NC_CACHE = None


def kernel(x, Wq, Wk, Wv, Wo, _trace=False, _tmpdir=None):
    global _NC_CACHE
    x = np.asarray(x, dtype=np.float32)
    Wq = np.asarray(Wq, dtype=np.float32)
    Wk = np.asarray(Wk, dtype=np.float32)
    Wv = np.asarray(Wv, dtype=np.float32)
    Wo = np.asarray(Wo, dtype=np.float32)

    if _NC_CACHE is None:
        _NC_CACHE = build_nc()
    nc = _NC_CACHE

    in_maps = make_in_maps(x, Wq, Wk, Wv, Wo)
    res = run_bass_kernel_spmd(nc, in_maps, core_ids=list(range(8)),
                               trace=_trace, tmpdir=_tmpdir)
    out = np.zeros((B, T, HID), dtype=np.float32)
    for c in range(8):
        out[c // 4] += res.results[c]["y"].astype(np.float32)
    if _trace:
        return out, res
    return out


# revision 27
# speedup vs baseline: 1.2130x; 1.0464x over previous
"""GQA attention (RoPE, causal) + output projection for Trainium2, 8 NeuronCores.

Problem: B=2, T=2048, HID=2048, NH=16 Q-heads, NKV=4 KV-heads, HD=128.
Sharding: tensor-parallel over the 4 KV-head groups (4 Q heads + 1 KV head per
group) x data-parallel over batch (2). Core c handles batch c//4, group c%4.
Each core computes its group's partial output y_g = A_g @ Wo[rows_g]; the
host unshards by summing the 4 row-parallel partials per batch.

The x shard is laid out transposed ([HID, T]) at shard-prep time so the
contraction dim lands on SBUF partitions without any on-device transposes.

Per-core device pipeline (all matmuls bf16, f32 accumulation in PSUM):
  A. Projections produce Q^T/K^T [d, t] directly (lhsT=W, rhs=xT) and
     V^T -> XBAR-transposed to natural [t, d]. RoPE is applied in [d, t]
     layout: rotate-half = two cross-partition DVE copies, tables arrive
     host-transposed; 1/sqrt(HD) is folded into the Q tables.
  B. Scores transposed: ST[kv,q] = matmul(lhsT=kT chunk, rhs=qT), exp on
     ScalarE (scores ~N(0,1): no max subtraction needed), multiplicative
     bf16 causal mask on diagonal tiles, then AT[d,q] += matmul(lhsT=V
     chunk, rhs=expST). Softmax sums via GpSimd partition-reduce of expST,
     reciprocal on DVE, GpSimd partition-broadcast, one DVE mul -> aT.
  C. y = A @ Wo via lhsT=aT slices, rhs=Wo; PSUM->SBUF copy on ScalarE.
"""

import numpy as np
import ml_dtypes

import concourse.bass as bass
import concourse.mybir as mybir
import concourse.tile as tile
from concourse import bacc
from concourse.bass_utils import run_bass_kernel_spmd

B, T, HID = 2, 2048, 2048
NH, NKV = 16, 4
HD = 128
GROUPS = NH // NKV      # 4 q-heads per kv head
NQ = GROUPS             # q heads per core
QW = NQ * HD            # 512 q cols per core
P = 128
TB = T // P             # 16 t-blocks
HC = HID // P           # 16 hid chunks
QS = T // 512           # 4 q supertiles
KVC = T // P            # 16 kv chunks
TS = T // 512           # 4 t supertiles
ROPE_BASE = 10000.0

F32 = mybir.dt.float32
BF16 = mybir.dt.bfloat16
EXP = mybir.ActivationFunctionType.Exp


def build_nc():
    nc = bacc.Bacc("TRN2", target_bir_lowering=False, debug=False,
                   enable_asserts=False, num_devices=8)

    xT_d = nc.dram_tensor("xT", [HID, T], F32, kind="ExternalInput")
    wq_d = nc.dram_tensor("wq", [HID, QW], F32, kind="ExternalInput")
    wk_d = nc.dram_tensor("wk", [HID, HD], F32, kind="ExternalInput")
    wv_d = nc.dram_tensor("wv", [HID, HD], F32, kind="ExternalInput")
    wo_d = nc.dram_tensor("wo", [QW, HID], F32, kind="ExternalInput")
    cosq_d = nc.dram_tensor("cosqT", [HD, T], BF16, kind="ExternalInput")
    sinq_d = nc.dram_tensor("sinqT", [HD, T], BF16, kind="ExternalInput")
    cosk_d = nc.dram_tensor("coskT", [HD, T], BF16, kind="ExternalInput")
    sink_d = nc.dram_tensor("sinkT", [HD, T], BF16, kind="ExternalInput")
    masks_d = nc.dram_tensor("masks", [P, P], BF16, kind="ExternalInput")
    y_d = nc.dram_tensor("y", [T, HID], BF16, kind="ExternalOutput")

    with tile.TileContext(nc) as tc:
        with tc.tile_pool(name="persist", bufs=1) as persist:
            # ---- persistent SBUF ----
            qT = persist.tile([P, NQ, T], BF16)        # (d, h, t)
            kT = persist.tile([P, T], BF16)            # (d, t)
            vnat = persist.tile([P, KVC, HD], BF16)    # (t, kvc, d)
            aT = persist.tile([P, NQ, T], BF16)        # (d, h, t)
            wq_s = persist.tile([P, HC, NQ, HD], BF16)
            wk_s = persist.tile([P, HC, HD], BF16)
            wv_s = persist.tile([P, HC, HD], BF16)
            wo_s = persist.tile([P, NQ, HID], BF16)
            cq_s = persist.tile([P, T], BF16)
            sq_s = persist.tile([P, T], BF16)
            ck_s = persist.tile([P, T], BF16)
            sk_s = persist.tile([P, T], BF16)
            masks_s = persist.tile([P, P], BF16)

            # ---- weights/constants load + cast (ordered so the first
            # projections' inputs arrive earliest: wk/wv -> tables -> wq) ----
            wpool_ctx = tc.tile_pool(name="stageW", bufs=1)
            stageW = wpool_ctx.__enter__()
            wqf = stageW.tile([P, HC, QW], F32, tag="wqf")
            nc.scalar.dma_start(wqf[:],
                                wq_d.ap().rearrange("(hc p) c -> p hc c", p=P))
            nc.vector.tensor_copy(
                wq_s.rearrange("p hc c d -> p hc (c d)"), wqf[:])
            wkf = stageW.tile([P, HC, HD], F32, tag="wkf")
            nc.scalar.dma_start(wkf[:], wk_d.ap().rearrange("(hc p) d -> p hc d", p=P))
            nc.vector.tensor_copy(wk_s[:], wkf[:])
            wvf = stageW.tile([P, HC, HD], F32, tag="wvf")
            nc.scalar.dma_start(wvf[:], wv_d.ap().rearrange("(hc p) d -> p hc d", p=P))
            nc.vector.tensor_copy(wv_s[:], wvf[:])
            nc.scalar.dma_start(masks_s[:], masks_d[:])
            nc.scalar.dma_start(cq_s[:], cosq_d[:])
            nc.scalar.dma_start(sq_s[:], sinq_d[:])
            nc.scalar.dma_start(ck_s[:], cosk_d[:])
            nc.scalar.dma_start(sk_s[:], sink_d[:])
            wpool_ctx.__exit__(None, None, None)

            # ---- stage A: projections + RoPE, per t-supertile ----
            with (
                tc.tile_pool(name="psA", bufs=2, space="PSUM") as psA,
                tc.tile_pool(name="stageA", bufs=3) as stageA,
            ):
                for ts in range(TS):
                    t0 = ts * 512
                    xts = stageA.tile([P, HC, 512], BF16, tag="xts", bufs=3)
                    for hq in range(4):
                        xf = stageA.tile([P, 4, 512], F32, tag="xf", bufs=3)
                        nc.sync.dma_start(
                            xf[:],
                            xT_d.ap()[hq * 4 * P:(hq + 1) * 4 * P, t0:t0 + 512]
                            .rearrange("(hc p) t -> p hc t", p=P))
                        nc.scalar.copy(
                            xts[:, hq * 4:(hq + 1) * 4].rearrange(
                                "p hc t -> p (hc t)"),
                            xf.rearrange("p hc t -> p (hc t)"))

                    def rope(ps, cs, ss, out_slice):
                        rot = stageA.tile([P, 512], F32, tag="rot", bufs=3)
                        nc.vector.tensor_copy(rot[0:64, :], ps[64:128, :])
                        nc.vector.tensor_copy(rot[64:128, :], ps[0:64, :])
                        qc = stageA.tile([P, 512], F32, tag="qc", bufs=3)
                        nc.vector.tensor_mul(qc[:], ps[:], cs)
                        nc.vector.tensor_mul(rot[:], rot[:], ss)
                        nc.vector.tensor_add(out_slice, qc[:], rot[:])

                    k_ps = psA.tile([P, 512], F32, tag="kps")
                    for hc in range(HC):
                        nc.tensor.matmul(k_ps[:], wk_s[:, hc], xts[:, hc],
                                         start=(hc == 0), stop=(hc == HC - 1))
                    rope(k_ps, ck_s[:, t0:t0 + 512], sk_s[:, t0:t0 + 512],
                         kT[:, t0:t0 + 512])
                    v_ps = psA.tile([P, 512], F32, tag="vps")
                    for hc in range(HC):
                        nc.tensor.matmul(v_ps[:], wv_s[:, hc], xts[:, hc],
                                         start=(hc == 0), stop=(hc == HC - 1))
                    vtb = stageA.tile([P, 512], BF16, tag="vtb", bufs=2)
                    nc.scalar.copy(vtb[:], v_ps[:])
                    for j in range(4):
                        nc.sync.dma_start_transpose(
                            vnat[:, ts * 4 + j, :], vtb[:, j * P:(j + 1) * P])
                    for h in range(NQ):
                        q_ps = psA.tile([P, 512], F32, tag="qps")
                        for hc in range(HC):
                            nc.tensor.matmul(q_ps[:], wq_s[:, hc, h],
                                             xts[:, hc],
                                             start=(hc == 0), stop=(hc == HC - 1))
                        rope(q_ps, cq_s[:, t0:t0 + 512], sq_s[:, t0:t0 + 512],
                             qT[:, h, t0:t0 + 512])
                    # wo chunk load overlapping later phases
                    wof = stageA.tile([P, HID], F32, tag="wof", bufs=2)
                    nc.scalar.dma_start(wof[:], wo_d[ts * P:(ts + 1) * P, :])
                    nc.vector.tensor_copy(wo_s[:, ts], wof[:])

            # ---- stage B: attention (heads in pairs to fit 8 PSUM banks) ----
            ones_s = persist.tile([P, P], BF16)
            nc.vector.memset(ones_s[:], 1.0)
            with (
                tc.tile_pool(name="psS", bufs=2, space="PSUM") as psS,
                tc.tile_pool(name="psAv", bufs=1, space="PSUM") as psAv,
                tc.tile_pool(name="stageB", bufs=3) as stageB,
            ):
                for qs in range(QS):
                    q0 = qs * 512
                    nkv = (qs + 1) * 4
                    for hp in range(2):
                        av = psAv.tile([P, 2, 512], F32, tag="av", bufs=1)
                        laccs = []
                        for hh in range(2):
                            lacc = stageB.tile([P, 512], BF16, bufs=2,
                                               tag=f"lacc{hh}",
                                               name=f"lacc{hh}")
                            laccs.append(lacc)
                        for kvc in range(nkv):
                            o = kvc - 4 * qs
                            c0 = max(o, 0) * P
                            psts = []
                            for hh in range(2):
                                h = hp * 2 + hh
                                st_ps = psS.tile([P, 512], F32, tag="st",
                                                 bufs=2)
                                nc.tensor.matmul(st_ps[:, c0:],
                                                 kT[:, kvc * P:(kvc + 1) * P],
                                                 qT[:, h, q0 + c0:q0 + 512],
                                                 start=True, stop=True)
                                pst = stageB.tile([P, 512], BF16, tag="pst",
                                                  bufs=4)
                                nc.scalar.activation(pst[:, c0:],
                                                     st_ps[:, c0:], EXP)
                                if o >= 0:
                                    nc.vector.tensor_mul(
                                        pst[:, c0:c0 + P], pst[:, c0:c0 + P],
                                        masks_s[:])
                                if kvc == 0:
                                    nc.vector.tensor_copy(laccs[hh][:], pst[:])
                                else:
                                    nc.vector.tensor_add(
                                        laccs[hh][:, c0:], laccs[hh][:, c0:],
                                        pst[:, c0:])
                                psts.append(pst)
                            for hh in range(2):
                                nc.tensor.matmul(av[:, hh, c0:],
                                                 vnat[:, kvc],
                                                 psts[hh][:, c0:],
                                                 start=(kvc == 0),
                                                 stop=(kvc == nkv - 1),
                                                 skip_group_check=True)
                        lb = psAv.tile([P, 2, 512], F32, tag="lb")
                        for hh in range(2):
                            nc.tensor.matmul(lb[:, hh], ones_s[:],
                                             laccs[hh][:],
                                             start=True, stop=True)
                        for hh in range(2):
                            h = hp * 2 + hh
                            rec = stageB.tile([P, 512], F32, tag="rec", bufs=2)
                            nc.vector.reciprocal_approx_fast(rec[:], lb[:, hh])
                            nc.vector.tensor_mul(aT[:, h, q0:q0 + 512],
                                                 av[:, hh], rec[:])
                    # ---- fused output projection for this q-supertile ----
                    for tb in range(4 * qs, 4 * qs + 4):
                        for npair in range(2):
                            yp0 = psAv.tile([P, 512], F32, tag="yps", bufs=2,
                                            name="yp0")
                            yp1 = psAv.tile([P, 512], F32, tag="yps", bufs=2,
                                            name="yp1")
                            yps = (yp0, yp1)
                            for cc in range(NQ):
                                for k in range(2):
                                    ns = 2 * npair + k
                                    nc.tensor.matmul(
                                        yps[k][:],
                                        aT[:, cc, tb * P:(tb + 1) * P],
                                        wo_s[:, cc, ns * 512:(ns + 1) * 512],
                                        start=(cc == 0), stop=(cc == NQ - 1))
                            for k in range(2):
                                ns = 2 * npair + k
                                y_sb = stageB.tile([P, 512], BF16, tag="ysb",
                                                   bufs=4)
                                if k == 0:
                                    nc.scalar.copy(y_sb[:], yps[k][:])
                                else:
                                    nc.vector.tensor_copy(y_sb[:], yps[k][:])
                                nc.scalar.dma_start(
                                    y_d[tb * P:(tb + 1) * P,
                                        ns * 512:(ns + 1) * 512],
                                    y_sb[:])

    nc.compile()
    return nc


def make_tables():
    inv_freq = 1.0 / (ROPE_BASE ** (np.arange(0, HD, 2, dtype=np.float64) / HD))
    t = np.arange(T, dtype=np.float64)
    freqs = np.outer(t, inv_freq)
    emb = np.concatenate([freqs, freqs], axis=-1)        # [T, 128]
    cos = np.cos(emb)
    sin = np.sin(emb)
    sin_signed = sin.copy()
    sin_signed[:, :64] = -sin_signed[:, :64]
    scale = 1.0 / np.sqrt(HD)
    bf = ml_dtypes.bfloat16
    cosqT = np.ascontiguousarray((cos * scale).T).astype(bf)
    sinqT = np.ascontiguousarray((sin_signed * scale).T).astype(bf)
    coskT = np.ascontiguousarray(cos.T).astype(bf)
    sinkT = np.ascontiguousarray(sin_signed.T).astype(bf)
    return cosqT, sinqT, coskT, sinkT


def make_masks():
    # triangle mask [kv=128, q=128]: 1 where kv_row <= q_col
    j = np.arange(P)[None, :]
    i = np.arange(P)[:, None]
    return (i <= j).astype(ml_dtypes.bfloat16)


def make_in_maps(x, Wq, Wk, Wv, Wo):
    cosqT, sinqT, coskT, sinkT = make_tables()
    masks = make_masks()
    in_maps = []
    for c in range(8):
        b, g = c // 4, c % 4
        in_maps.append({
            "xT": np.ascontiguousarray(x[b].T),
            "wq": np.ascontiguousarray(Wq[:, g * QW:(g + 1) * QW]),
            "wk": np.ascontiguousarray(Wk[:, g * HD:(g + 1) * HD]),
            "wv": np.ascontiguousarray(Wv[:, g * HD:(g + 1) * HD]),
            "wo": np.ascontiguousarray(Wo[g * QW:(g + 1) * QW, :]),
            "cosqT": cosqT, "sinqT": sinqT, "coskT": coskT, "sinkT": sinkT,
            "masks": masks,
        })
    return in_maps


_NC_CACHE = None


def kernel(x, Wq, Wk, Wv, Wo, _trace=False, _tmpdir=None):
    global _NC_CACHE
    x = np.asarray(x, dtype=np.float32)
    Wq = np.asarray(Wq, dtype=np.float32)
    Wk = np.asarray(Wk, dtype=np.float32)
    Wv = np.asarray(Wv, dtype=np.float32)
    Wo = np.asarray(Wo, dtype=np.float32)

    if _NC_CACHE is None:
        _NC_CACHE = build_nc()
    nc = _NC_CACHE

    in_maps = make_in_maps(x, Wq, Wk, Wv, Wo)
    res = run_bass_kernel_spmd(nc, in_maps, core_ids=list(range(8)),
                               trace=_trace, tmpdir=_tmpdir)
    out = np.zeros((B, T, HID), dtype=np.float32)
    for c in range(8):
        out[c // 4] += res.results[c]["y"].astype(np.float32)
    if _trace:
        return out, res
    return out


# revision 28
# speedup vs baseline: 1.2216x; 1.0071x over previous
"""GQA attention (RoPE, causal) + output projection for Trainium2, 8 NeuronCores.

Problem: B=2, T=2048, HID=2048, NH=16 Q-heads, NKV=4 KV-heads, HD=128.
Sharding: tensor-parallel over the 4 KV-head groups (4 Q heads + 1 KV head per
group) x data-parallel over batch (2). Core c handles batch c//4, group c%4.
Each core computes its group's partial output y_g = A_g @ Wo[rows_g]; the
host unshards by summing the 4 row-parallel partials per batch.

The x shard is laid out transposed ([HID, T]) at shard-prep time so the
contraction dim lands on SBUF partitions without any on-device transposes.

Per-core device pipeline (all matmuls bf16, f32 accumulation in PSUM):
  A. Projections produce Q^T/K^T [d, t] directly (lhsT=W, rhs=xT) and
     V^T -> XBAR-transposed to natural [t, d]. RoPE is applied in [d, t]
     layout: rotate-half = two cross-partition DVE copies, tables arrive
     host-transposed; 1/sqrt(HD) is folded into the Q tables.
  B. Scores transposed: ST[kv,q] = matmul(lhsT=kT chunk, rhs=qT), exp on
     ScalarE (scores ~N(0,1): no max subtraction needed), multiplicative
     bf16 causal mask on diagonal tiles, then AT[d,q] += matmul(lhsT=V
     chunk, rhs=expST). Softmax sums via GpSimd partition-reduce of expST,
     reciprocal on DVE, GpSimd partition-broadcast, one DVE mul -> aT.
  C. y = A @ Wo via lhsT=aT slices, rhs=Wo; PSUM->SBUF copy on ScalarE.
"""

import numpy as np
import ml_dtypes

import concourse.bass as bass
import concourse.mybir as mybir
import concourse.tile as tile
from concourse import bacc
from concourse.bass_utils import run_bass_kernel_spmd

B, T, HID = 2, 2048, 2048
NH, NKV = 16, 4
HD = 128
GROUPS = NH // NKV      # 4 q-heads per kv head
NQ = GROUPS             # q heads per core
QW = NQ * HD            # 512 q cols per core
P = 128
TB = T // P             # 16 t-blocks
HC = HID // P           # 16 hid chunks
QS = T // 512           # 4 q supertiles
KVC = T // P            # 16 kv chunks
TS = T // 512           # 4 t supertiles
ROPE_BASE = 10000.0

F32 = mybir.dt.float32
BF16 = mybir.dt.bfloat16
EXP = mybir.ActivationFunctionType.Exp


def build_nc():
    nc = bacc.Bacc("TRN2", target_bir_lowering=False, debug=False,
                   enable_asserts=False, num_devices=8)

    xT_d = nc.dram_tensor("xT", [HID, T], F32, kind="ExternalInput")
    wq_d = nc.dram_tensor("wq", [P, HC, QW], F32, kind="ExternalInput")
    wk_d = nc.dram_tensor("wk", [P, HC, HD], F32, kind="ExternalInput")
    wv_d = nc.dram_tensor("wv", [P, HC, HD], F32, kind="ExternalInput")
    wo_d = nc.dram_tensor("wo", [QW, HID], F32, kind="ExternalInput")
    cosq_d = nc.dram_tensor("cosqT", [HD, T], BF16, kind="ExternalInput")
    sinq_d = nc.dram_tensor("sinqT", [HD, T], BF16, kind="ExternalInput")
    cosk_d = nc.dram_tensor("coskT", [HD, T], BF16, kind="ExternalInput")
    sink_d = nc.dram_tensor("sinkT", [HD, T], BF16, kind="ExternalInput")
    masks_d = nc.dram_tensor("masks", [P, P], BF16, kind="ExternalInput")
    y_d = nc.dram_tensor("y", [T, HID], BF16, kind="ExternalOutput")

    with tile.TileContext(nc) as tc:
        with tc.tile_pool(name="persist", bufs=1) as persist:
            # ---- persistent SBUF ----
            qT = persist.tile([P, NQ, T], BF16)        # (d, h, t)
            kT = persist.tile([P, T], BF16)            # (d, t)
            vnat = persist.tile([P, KVC, HD], BF16)    # (t, kvc, d)
            aT = persist.tile([P, NQ, T], BF16)        # (d, h, t)
            wq_s = persist.tile([P, HC, NQ, HD], BF16)
            wk_s = persist.tile([P, HC, HD], BF16)
            wv_s = persist.tile([P, HC, HD], BF16)
            wo_s = persist.tile([P, NQ, HID], BF16)
            cq_s = persist.tile([P, T], BF16)
            sq_s = persist.tile([P, T], BF16)
            ck_s = persist.tile([P, T], BF16)
            sk_s = persist.tile([P, T], BF16)
            masks_s = persist.tile([P, P], BF16)

            # ---- weights/constants load + cast (ordered so the first
            # projections' inputs arrive earliest: wk/wv -> tables -> wq) ----
            wpool_ctx = tc.tile_pool(name="stageW", bufs=1)
            stageW = wpool_ctx.__enter__()
            wqf = stageW.tile([P, HC, QW], F32, tag="wqf")
            nc.scalar.dma_start(wqf[:], wq_d[:])
            nc.vector.tensor_copy(
                wq_s.rearrange("p hc c d -> p hc (c d)"), wqf[:])
            wkf = stageW.tile([P, HC, HD], F32, tag="wkf")
            nc.scalar.dma_start(wkf[:], wk_d[:])
            nc.vector.tensor_copy(wk_s[:], wkf[:])
            wvf = stageW.tile([P, HC, HD], F32, tag="wvf")
            nc.scalar.dma_start(wvf[:], wv_d[:])
            nc.vector.tensor_copy(wv_s[:], wvf[:])
            nc.scalar.dma_start(masks_s[:], masks_d[:])
            nc.scalar.dma_start(cq_s[:], cosq_d[:])
            nc.scalar.dma_start(sq_s[:], sinq_d[:])
            nc.scalar.dma_start(ck_s[:], cosk_d[:])
            nc.scalar.dma_start(sk_s[:], sink_d[:])
            wpool_ctx.__exit__(None, None, None)

            # ---- stage A: projections + RoPE, per t-supertile ----
            with (
                tc.tile_pool(name="psA", bufs=2, space="PSUM") as psA,
                tc.tile_pool(name="stageA", bufs=3) as stageA,
            ):
                for ts in range(TS):
                    t0 = ts * 512
                    xts = stageA.tile([P, HC, 512], BF16, tag="xts", bufs=3)
                    for hq in range(4):
                        xf = stageA.tile([P, 4, 512], F32, tag="xf", bufs=3)
                        nc.sync.dma_start(
                            xf[:],
                            xT_d.ap()[hq * 4 * P:(hq + 1) * 4 * P, t0:t0 + 512]
                            .rearrange("(hc p) t -> p hc t", p=P))
                        nc.scalar.copy(
                            xts[:, hq * 4:(hq + 1) * 4].rearrange(
                                "p hc t -> p (hc t)"),
                            xf.rearrange("p hc t -> p (hc t)"))

                    def rope(ps, cs, ss, out_slice):
                        rot = stageA.tile([P, 512], F32, tag="rot", bufs=3)
                        nc.vector.tensor_copy(rot[0:64, :], ps[64:128, :])
                        nc.vector.tensor_copy(rot[64:128, :], ps[0:64, :])
                        qc = stageA.tile([P, 512], F32, tag="qc", bufs=3)
                        nc.vector.tensor_mul(qc[:], ps[:], cs)
                        nc.vector.tensor_mul(rot[:], rot[:], ss)
                        nc.vector.tensor_add(out_slice, qc[:], rot[:])

                    k_ps = psA.tile([P, 512], F32, tag="kps")
                    for hc in range(HC):
                        nc.tensor.matmul(k_ps[:], wk_s[:, hc], xts[:, hc],
                                         start=(hc == 0), stop=(hc == HC - 1))
                    rope(k_ps, ck_s[:, t0:t0 + 512], sk_s[:, t0:t0 + 512],
                         kT[:, t0:t0 + 512])
                    v_ps = psA.tile([P, 512], F32, tag="vps")
                    for hc in range(HC):
                        nc.tensor.matmul(v_ps[:], wv_s[:, hc], xts[:, hc],
                                         start=(hc == 0), stop=(hc == HC - 1))
                    vtb = stageA.tile([P, 512], BF16, tag="vtb", bufs=2)
                    nc.scalar.copy(vtb[:], v_ps[:])
                    for j in range(4):
                        nc.sync.dma_start_transpose(
                            vnat[:, ts * 4 + j, :], vtb[:, j * P:(j + 1) * P])
                    for h in range(NQ):
                        q_ps = psA.tile([P, 512], F32, tag="qps")
                        for hc in range(HC):
                            nc.tensor.matmul(q_ps[:], wq_s[:, hc, h],
                                             xts[:, hc],
                                             start=(hc == 0), stop=(hc == HC - 1))
                        rope(q_ps, cq_s[:, t0:t0 + 512], sq_s[:, t0:t0 + 512],
                             qT[:, h, t0:t0 + 512])
                    # wo chunk load overlapping later phases
                    wof = stageA.tile([P, HID], F32, tag="wof", bufs=2)
                    nc.scalar.dma_start(wof[:], wo_d[ts * P:(ts + 1) * P, :])
                    nc.vector.tensor_copy(wo_s[:, ts], wof[:])

            # ---- stage B: attention (heads in pairs to fit 8 PSUM banks) ----
            ones_s = persist.tile([P, P], BF16)
            nc.vector.memset(ones_s[:], 1.0)
            with (
                tc.tile_pool(name="psS", bufs=2, space="PSUM") as psS,
                tc.tile_pool(name="psAv", bufs=1, space="PSUM") as psAv,
                tc.tile_pool(name="stageB", bufs=3) as stageB,
            ):
                for qs in range(QS):
                    q0 = qs * 512
                    nkv = (qs + 1) * 4
                    for hp in range(2):
                        av = psAv.tile([P, 2, 512], F32, tag="av", bufs=1)
                        laccs = []
                        for hh in range(2):
                            lacc = stageB.tile([P, 512], BF16, bufs=2,
                                               tag=f"lacc{hh}",
                                               name=f"lacc{hh}")
                            laccs.append(lacc)
                        for kvc in range(nkv):
                            o = kvc - 4 * qs
                            c0 = max(o, 0) * P
                            psts = []
                            for hh in range(2):
                                h = hp * 2 + hh
                                st_ps = psS.tile([P, 512], F32, tag="st",
                                                 bufs=2)
                                nc.tensor.matmul(st_ps[:, c0:],
                                                 kT[:, kvc * P:(kvc + 1) * P],
                                                 qT[:, h, q0 + c0:q0 + 512],
                                                 start=True, stop=True)
                                pst = stageB.tile([P, 512], BF16, tag="pst",
                                                  bufs=4)
                                nc.scalar.activation(pst[:, c0:],
                                                     st_ps[:, c0:], EXP)
                                if o >= 0:
                                    nc.vector.tensor_mul(
                                        pst[:, c0:c0 + P], pst[:, c0:c0 + P],
                                        masks_s[:])
                                if kvc == 0:
                                    nc.vector.tensor_copy(laccs[hh][:], pst[:])
                                else:
                                    nc.vector.tensor_add(
                                        laccs[hh][:, c0:], laccs[hh][:, c0:],
                                        pst[:, c0:])
                                psts.append(pst)
                            for hh in range(2):
                                nc.tensor.matmul(av[:, hh, c0:],
                                                 vnat[:, kvc],
                                                 psts[hh][:, c0:],
                                                 start=(kvc == 0),
                                                 stop=(kvc == nkv - 1),
                                                 skip_group_check=True)
                        lb = psAv.tile([P, 2, 512], F32, tag="lb")
                        for hh in range(2):
                            nc.tensor.matmul(lb[:, hh], ones_s[:],
                                             laccs[hh][:],
                                             start=True, stop=True)
                        for hh in range(2):
                            h = hp * 2 + hh
                            rec = stageB.tile([P, 512], F32, tag="rec", bufs=2)
                            nc.vector.reciprocal_approx_fast(rec[:], lb[:, hh])
                            nc.vector.tensor_mul(aT[:, h, q0:q0 + 512],
                                                 av[:, hh], rec[:])
                    # ---- fused output projection for this q-supertile ----
                    for tb in range(4 * qs, 4 * qs + 4):
                        for npair in range(2):
                            yp0 = psAv.tile([P, 512], F32, tag="yps", bufs=2,
                                            name="yp0")
                            yp1 = psAv.tile([P, 512], F32, tag="yps", bufs=2,
                                            name="yp1")
                            yps = (yp0, yp1)
                            for cc in range(NQ):
                                for k in range(2):
                                    ns = 2 * npair + k
                                    nc.tensor.matmul(
                                        yps[k][:],
                                        aT[:, cc, tb * P:(tb + 1) * P],
                                        wo_s[:, cc, ns * 512:(ns + 1) * 512],
                                        start=(cc == 0), stop=(cc == NQ - 1))
                            for k in range(2):
                                ns = 2 * npair + k
                                y_sb = stageB.tile([P, 512], BF16, tag="ysb",
                                                   bufs=4)
                                if k == 0:
                                    nc.scalar.copy(y_sb[:], yps[k][:])
                                else:
                                    nc.vector.tensor_copy(y_sb[:], yps[k][:])
                                nc.scalar.dma_start(
                                    y_d[tb * P:(tb + 1) * P,
                                        ns * 512:(ns + 1) * 512],
                                    y_sb[:])

    nc.compile()
    return nc


def make_tables():
    inv_freq = 1.0 / (ROPE_BASE ** (np.arange(0, HD, 2, dtype=np.float64) / HD))
    t = np.arange(T, dtype=np.float64)
    freqs = np.outer(t, inv_freq)
    emb = np.concatenate([freqs, freqs], axis=-1)        # [T, 128]
    cos = np.cos(emb)
    sin = np.sin(emb)
    sin_signed = sin.copy()
    sin_signed[:, :64] = -sin_signed[:, :64]
    scale = 1.0 / np.sqrt(HD)
    bf = ml_dtypes.bfloat16
    cosqT = np.ascontiguousarray((cos * scale).T).astype(bf)
    sinqT = np.ascontiguousarray((sin_signed * scale).T).astype(bf)
    coskT = np.ascontiguousarray(cos.T).astype(bf)
    sinkT = np.ascontiguousarray(sin_signed.T).astype(bf)
    return cosqT, sinqT, coskT, sinkT


def make_masks():
    # triangle mask [kv=128, q=128]: 1 where kv_row <= q_col
    j = np.arange(P)[None, :]
    i = np.arange(P)[:, None]
    return (i <= j).astype(ml_dtypes.bfloat16)


def make_in_maps(x, Wq, Wk, Wv, Wo):
    cosqT, sinqT, coskT, sinkT = make_tables()
    masks = make_masks()
    in_maps = []
    for c in range(8):
        b, g = c // 4, c % 4
        in_maps.append({
            "xT": np.ascontiguousarray(x[b].T),
            "wq": np.ascontiguousarray(
                Wq[:, g * QW:(g + 1) * QW].reshape(HC, P, QW)
                .transpose(1, 0, 2)),
            "wk": np.ascontiguousarray(
                Wk[:, g * HD:(g + 1) * HD].reshape(HC, P, HD)
                .transpose(1, 0, 2)),
            "wv": np.ascontiguousarray(
                Wv[:, g * HD:(g + 1) * HD].reshape(HC, P, HD)
                .transpose(1, 0, 2)),
            "wo": np.ascontiguousarray(Wo[g * QW:(g + 1) * QW, :]),
            "cosqT": cosqT, "sinqT": sinqT, "coskT": coskT, "sinkT": sinkT,
            "masks": masks,
        })
    return in_maps


_NC_CACHE = None


def kernel(x, Wq, Wk, Wv, Wo, _trace=False, _tmpdir=None):
    global _NC_CACHE
    x = np.asarray(x, dtype=np.float32)
    Wq = np.asarray(Wq, dtype=np.float32)
    Wk = np.asarray(Wk, dtype=np.float32)
    Wv = np.asarray(Wv, dtype=np.float32)
    Wo = np.asarray(Wo, dtype=np.float32)

    if _NC_CACHE is None:
        _NC_CACHE = build_nc()
    nc = _NC_CACHE

    in_maps = make_in_maps(x, Wq, Wk, Wv, Wo)
    res = run_bass_kernel_spmd(nc, in_maps, core_ids=list(range(8)),
                               trace=_trace, tmpdir=_tmpdir)
    out = np.zeros((B, T, HID), dtype=np.float32)
    for c in range(8):
        out[c // 4] += res.results[c]["y"].astype(np.float32)
    if _trace:
        return out, res
    return out
